# revision 1
# baseline (speedup 1.0000x reference)
"""Deformable conv block on 8 TRN2 NeuronCores (Bass/Tile).

Sharding: (batch, H-half) -> 8 cores. Each core computes 48 output rows
(all 128 channels) of one batch image.

Per-core pipeline (all fp16 matmuls, fp32 PSUM):
  1. conv1 est = relu(conv3x3([lr;hr]))   : shifted-window matmuls, fp16
  2. conv2 offset = conv3x3(est) -> [18, 4608] fp32
  3. offset -> sample positions: clamp to [-2, 96] (exact: all-invalid taps
     stay zero via the zero-padded gather image), floor via +1.5*2^23 trick,
     bilinear weights; gather indices into wrapped+replicated int16 layout
  4. dma_gather 4-corner quads (1KB/px/tap) from a host-prebuilt duplicated
     pixel-major fp16 image; per-partition scalar_tensor_tensor bilinear
     combine; PE transpose back to channel-major; 3x3 deform conv matmuls.
"""
import sys

sys.path.insert(0, "/opt/trn_rl_repo")

import numpy as np

import concourse.bacc as bacc
import concourse.bass as bass
import concourse.mybir as mybir
from concourse.tile import TileContext
from concourse.masks import make_identity
from concourse.library_config import mlp
from concourse.bass_utils import run_bass_kernel_spmd

F32 = mybir.dt.float32
F16 = mybir.dt.float16
I16 = mybir.dt.int16

B, C, H, W = 4, 128, 96, 96
NPIX = 48 * 96                 # pixels per core (half image)
HP, WP = 52, 98                # conv input slice: 48+2*2 rows, 96+2 cols
PW = 100                       # padded gather image width/height
NDUP = 10112                   # dup2 rows (>= 10001, mult of 128)
NT = 9
NBLK = NPIX // 128             # 36 pixel blocks of 128
GCH = 2304                     # gather chunk (pixels per dma_gather)
SUB = 384                      # dc matmul pixel sub-chunk (psum N)
MAGIC = float(3 * 2 ** 22)     # RNE-integer rounding constant


def _build_nc(dbg=0, reps=1):
    nc = bacc.Bacc("TRN2", target_bir_lowering=False, debug=False)

    lr_pad = nc.dram_tensor("lr_pad", [C, HP, WP], F16, kind="ExternalInput")
    hr_cpad = nc.dram_tensor("hr_cpad", [C, HP, WP], F16, kind="ExternalInput")
    hr_dup = nc.dram_tensor("hr_dup", [NDUP * 256], F16, kind="ExternalInput")
    w_est = nc.dram_tensor("w_est", [2, NT, C, 2 * C], F16, kind="ExternalInput")
    w_off = nc.dram_tensor("w_off", [2, NT, C, 18], F16, kind="ExternalInput")
    w_dc = nc.dram_tensor("w_dc", [NT, C, C], F16, kind="ExternalInput")
    est_b = nc.dram_tensor("est_b", [C, 2], F32, kind="ExternalInput")
    off_b = nc.dram_tensor("off_b", [18, 1], F32, kind="ExternalInput")
    dc_b = nc.dram_tensor("dc_b", [C, 1], F32, kind="ExternalInput")
    base_y = nc.dram_tensor("base_y", [C, NT, NBLK], F32, kind="ExternalInput")
    base_x = nc.dram_tensor("base_x", [C, NT, NBLK], F32, kind="ExternalInput")
    mask_e = nc.dram_tensor("mask_e", [C, 2], F32, kind="ExternalInput")

    out = nc.dram_tensor("out", [C, NPIX], F32, kind="ExternalOutput")
    if dbg:
        d_off = nc.dram_tensor("d_off", [18, NPIX], F32, kind="ExternalOutput")
        d_idx = nc.dram_tensor("d_idx", [C, NT * NBLK], I16, kind="ExternalOutput")
        d_w = nc.dram_tensor("d_w", [4, C, NT * NBLK], F32, kind="ExternalOutput")
    off_dram = nc.dram_tensor("off_dram", [18 * NPIX], F32)
    idx_dram = nc.dram_tensor("idx_dram", [NT * NPIX], I16)

    with TileContext(nc) as tc:
        with tc.tile_pool(name="wsb", bufs=1) as wsb, \
             tc.tile_pool(name="img", bufs=1) as img, \
             tc.tile_pool(name="estp", bufs=1) as estp, \
             tc.tile_pool(name="math", bufs=1) as mathp, \
             tc.tile_pool(name="qpool", bufs=3) as qpool, \
             tc.tile_pool(name="vpool", bufs=6) as vpool, \
             tc.tile_pool(name="rhsp", bufs=1) as rhsp, \
             tc.tile_pool(name="outp", bufs=4) as outp, \
             tc.tile_pool(name="ps_conv", bufs=2, space="PSUM") as ps_conv, \
             tc.tile_pool(name="ps_off", bufs=1, space="PSUM") as ps_off, \
             tc.tile_pool(name="ps_val", bufs=2, space="PSUM") as ps_val, \
             tc.tile_pool(name="ps_out", bufs=3, space="PSUM") as ps_out:

            # ---------------- loads ----------------
            lr_sb = img.tile([C, HP * WP], F16)
            nc.sync.dma_start(lr_sb, lr_pad.reshape([C, HP * WP])[:, :])
            hr_sb = img.tile([C, HP * WP], F16)
            nc.sync.dma_start(hr_sb, hr_cpad.reshape([C, HP * WP])[:, :])
            w_est_sb = wsb.tile([C, 2, NT, 2 * C], F16)
            nc.sync.dma_start(
                w_est_sb,
                bass.AP(w_est.reshape([2 * NT * C * 2 * C])[:].tensor, 0,
                        [[2 * C, C], [NT * C * 2 * C, 2], [C * 2 * C, NT],
                         [1, 2 * C]]))
            w_off_sb = wsb.tile([C, 2, NT, 18], F16)
            nc.sync.dma_start(
                w_off_sb,
                bass.AP(w_off.reshape([2 * NT * C * 18])[:].tensor, 0,
                        [[18, C], [NT * C * 18, 2], [C * 18, NT], [1, 18]]))
            w_dc_sb = wsb.tile([C, NT, C], F16)
            nc.sync.dma_start(
                w_dc_sb,
                bass.AP(w_dc.reshape([NT * C * C])[:].tensor, 0,
                        [[C, C], [C * C, NT], [1, C]]))
            est_b_sb = wsb.tile([C, 2], F32)
            nc.sync.dma_start(est_b_sb, est_b[:, :])
            off_b_sb = wsb.tile([18, 1], F32)
            nc.sync.dma_start(off_b_sb, off_b[:, :])
            dc_b_sb = wsb.tile([C, 1], F32)
            nc.sync.dma_start(dc_b_sb, dc_b[:, :])
            by_sb = mathp.tile([C, NT, NBLK], F32)
            nc.sync.dma_start(by_sb, base_y[:, :, :])
            bx_sb = mathp.tile([C, NT, NBLK], F32)
            nc.sync.dma_start(bx_sb, base_x[:, :, :])
            mask_sb = wsb.tile([C, 2], F32)
            nc.sync.dma_start(mask_sb, mask_e[:, :])
            ident = wsb.tile([128, 128], F16)
            make_identity(nc, ident)
            ident32 = wsb.tile([32, 32], F32)
            make_identity(nc, ident32)

            # ---------------- conv1: est ----------------
            for rep in range(reps):
              est_sb = [estp.tile([C, HP * WP], F16, tag=f"est{i}", name=f"est{i}_{rep}")
                        for i in range(2)]
              for cb in range(2):
                  # zero the 1-px borders the conv2 reads
                  nc.vector.memset(bass.AP(est_sb[cb].tensor, est_sb[cb].offset,
                                           [est_sb[cb].ap[0], [1, WP]]), 0.0)
                  nc.vector.memset(bass.AP(est_sb[cb].tensor,
                                           est_sb[cb].offset + 51 * WP,
                                           [est_sb[cb].ap[0], [1, WP]]), 0.0)
                  nc.vector.memset(bass.AP(est_sb[cb].tensor, est_sb[cb].offset,
                                           [est_sb[cb].ap[0], [WP, HP], [1, 1]]), 0.0)
                  nc.vector.memset(bass.AP(est_sb[cb].tensor,
                                           est_sb[cb].offset + WP - 1,
                                           [est_sb[cb].ap[0], [WP, HP], [1, 1]]), 0.0)
              for cb_out in range(2):
                  for e0 in range(0, 50, 5):
                      ps = ps_conv.tile([C, 480], F32)
                      n = 0
                      for cin in range(2):
                          src = lr_sb if cin == 0 else hr_sb
                          for t in range(NT):
                              ky, kx = t // 3 - 1, t % 3 - 1
                              rhs = bass.AP(src.tensor,
                                            src.offset + (e0 + 1 + ky) * WP + kx + 1,
                                            [src.ap[0], [WP, 5], [1, 96]])
                              lhsT = w_est_sb[:, cin, t, cb_out * C:(cb_out + 1) * C]
                              nc.tensor.matmul(ps[:, :], lhsT, rhs,
                                               start=(n == 0), stop=(n == 17))
                              n += 1
                      dst = bass.AP(est_sb[cb_out].tensor,
                                    est_sb[cb_out].offset + (e0 + 1) * WP + 1,
                                    [est_sb[cb_out].ap[0], [WP, 5], [1, 96]])
                      nc.scalar.activation(dst, ps[:, :],
                                           mybir.ActivationFunctionType.Relu,
                                           bias=est_b_sb[:, cb_out:cb_out + 1])
                  # rows s=1 (global h0-1) / s=50 (global h0+48) may fall outside
                  # the image; conv2's zero padding requires them to be zero there
                  for si, s in enumerate((1, 50)):
                      nc.vector.tensor_scalar(
                          out=est_sb[cb_out][:, s * WP:(s + 1) * WP],
                          in0=est_sb[cb_out][:, s * WP:(s + 1) * WP],
                          scalar1=mask_sb[:, si:si + 1], scalar2=None,
                          op0=mybir.AluOpType.mult)

              # ---------------- conv2: offset ----------------
              off_sb = mathp.tile([18, NPIX], F32)
              for o0 in range(0, 48, 4):
                  ps = ps_off.tile([18, SUB], F32)
                  n = 0
                  for cin in range(2):
                      for t in range(NT):
                          ky, kx = t // 3 - 1, t % 3 - 1
                          src = est_sb[cin]
                          rhs = bass.AP(src.tensor,
                                        src.offset + (o0 + 2 + ky) * WP + kx + 1,
                                        [src.ap[0], [WP, 4], [1, 96]])
                          nc.tensor.matmul(ps[:, :], w_off_sb[:, cin, t, :], rhs,
                                           start=(n == 0), stop=(n == 17))
                          n += 1
                  nc.scalar.activation(off_sb[:, o0 * 96:(o0 + 4) * 96], ps[:, :],
                                       mybir.ActivationFunctionType.Identity,
                                       bias=off_b_sb[:, :])

              # ---------------- idx & weight math ----------------
              # pixel-major offsets via PE transposes (no DRAM bounce)
              TAP = NT * NBLK  # 324
              off_pm = mathp.tile([C, NBLK, 18], F32)
              for j in range(NBLK):
                  pst = ps_out.tile([C, 18], F32, tag="po")
                  nc.tensor.transpose(pst[:, :], off_sb[:, j * 128:(j + 1) * 128],
                                      ident32[0:18, 0:18])
                  nc.scalar.activation(off_pm[:, j, :], pst[:, :],
                                       mybir.ActivationFunctionType.Copy)
              dy = bass.AP(off_pm.tensor, off_pm.offset,
                           [off_pm.ap[0], [2, NT], [18, NBLK]])
              dx = bass.AP(off_pm.tensor, off_pm.offset + 1,
                           [off_pm.ap[0], [2, NT], [18, NBLK]])

              def floor_frac(v, ax):
                  """returns (floor(v), frac(v)) tiles, v in [-2, 96]"""
                  t_ = mathp.tile([C, TAP], F32, tag="ff_t")
                  nc.vector.tensor_scalar(out=t_[:, :], in0=v[:, :], scalar1=MAGIC,
                                          scalar2=MAGIC, op0=mybir.AluOpType.add,
                                          op1=mybir.AluOpType.subtract)
                  g_ = mathp.tile([C, TAP], F32, tag="ff_g")
                  nc.vector.tensor_tensor(out=g_[:, :], in0=t_[:, :], in1=v[:, :],
                                          op=mybir.AluOpType.is_gt)
                  fl = mathp.tile([C, TAP], F32, tag="ff_fl" + ax)
                  nc.vector.tensor_tensor(out=fl[:, :], in0=t_[:, :], in1=g_[:, :],
                                          op=mybir.AluOpType.subtract)
                  fr = mathp.tile([C, TAP], F32, tag="ff_fr" + ax)
                  nc.vector.tensor_tensor(out=fr[:, :], in0=v[:, :], in1=fl[:, :],
                                          op=mybir.AluOpType.subtract)
                  return fl, fr

              py = mathp.tile([C, TAP], F32)
              nc.vector.tensor_tensor(out=py[:, :], in0=dy[:, :],
                                      in1=bass.AP(by_sb.tensor, by_sb.offset, [by_sb.ap[0], [1, TAP]]),
                                      op=mybir.AluOpType.add)
              nc.vector.tensor_scalar(out=py[:, :], in0=py[:, :], scalar1=-2.0,
                                      scalar2=96.0, op0=mybir.AluOpType.max,
                                      op1=mybir.AluOpType.min)
              px = mathp.tile([C, TAP], F32)
              nc.vector.tensor_tensor(out=px[:, :], in0=dx[:, :],
                                      in1=bass.AP(bx_sb.tensor, bx_sb.offset, [bx_sb.ap[0], [1, TAP]]),
                                      op=mybir.AluOpType.add)
              nc.vector.tensor_scalar(out=px[:, :], in0=px[:, :], scalar1=-2.0,
                                      scalar2=96.0, op0=mybir.AluOpType.max,
                                      op1=mybir.AluOpType.min)
              y0, ly = floor_frac(py, "y")
              x0, lx = floor_frac(px, "x")

              # bilinear corner weights, pixel-major [C, NT, NBLK]
              wy0 = mathp.tile([C, TAP], F32)
              nc.vector.tensor_scalar(out=wy0[:, :], in0=ly[:, :], scalar1=-1.0,
                                      scalar2=1.0, op0=mybir.AluOpType.mult,
                                      op1=mybir.AluOpType.add)
              wx0 = mathp.tile([C, TAP], F32)
              nc.vector.tensor_scalar(out=wx0[:, :], in0=lx[:, :], scalar1=-1.0,
                                      scalar2=1.0, op0=mybir.AluOpType.mult,
                                      op1=mybir.AluOpType.add)
              w00 = mathp.tile([C, TAP], F32)
              nc.vector.tensor_tensor(out=w00[:, :], in0=wy0[:, :], in1=wx0[:, :],
                                      op=mybir.AluOpType.mult)
              w10 = mathp.tile([C, TAP], F32)
              nc.vector.tensor_tensor(out=w10[:, :], in0=ly[:, :], in1=wx0[:, :],
                                      op=mybir.AluOpType.mult)
              w01 = mathp.tile([C, TAP], F32)
              nc.vector.tensor_tensor(out=w01[:, :], in0=wy0[:, :], in1=lx[:, :],
                                      op=mybir.AluOpType.mult)
              w11 = mathp.tile([C, TAP], F32)
              nc.vector.tensor_tensor(out=w11[:, :], in0=ly[:, :], in1=lx[:, :],
                                      op=mybir.AluOpType.mult)

              # gather index: (y0+2)*100 + (x0+2) = 100*y0 + x0 + 202
              idxf = mathp.tile([C, TAP], F32)
              nc.vector.scalar_tensor_tensor(out=idxf[:, :], in0=y0[:, :],
                                             scalar=100.0, in1=x0[:, :],
                                             op0=mybir.AluOpType.mult,
                                             op1=mybir.AluOpType.add)
              nc.vector.tensor_scalar(out=idxf[:, :], in0=idxf[:, :], scalar1=202.0,
                                      scalar2=None, op0=mybir.AluOpType.add)
              idx16 = mathp.tile([C, TAP], I16)
              nc.vector.tensor_copy(idx16[:, :], idxf[:, :])
              for t in range(NT):
                  nc.sync.dma_start(
                      bass.AP(idx_dram[:].tensor, t * NPIX, [[NBLK, 128], [1, NBLK]]),
                      idx16[:, t * NBLK:(t + 1) * NBLK])
              tc.strict_bb_all_engine_barrier()
              idx_wt = [mathp.tile([C, NPIX // 16], I16, tag=f"idxw{t}",
                                   name=f"idxw{t}_{rep}") for t in range(NT)]
              for t in range(NT):
                  nc.sync.dma_start(
                      idx_wt[t][0:16, :],
                      bass.AP(idx_dram[:].tensor, t * NPIX,
                              [[36, 16], [1, 36], [16 * 36, 8]]))
                  for g in range(1, 8):
                      nc.sync.dma_start(idx_wt[t][16 * g:16 * (g + 1), :],
                                        idx_wt[t][0:16, :])

              if dbg:
                  nc.sync.dma_start(d_off[:, :], off_sb[:, :])
                  nc.sync.dma_start(d_idx[:, :], idx16[:, :])
                  for wi, wt in enumerate((w00, w10, w01, w11)):
                      nc.sync.dma_start(d_w[wi, :, :], wt[:, :])
              # ---------------- gather + combine + dc conv ----------------
              dup_ap_tensor = hr_dup[:].tensor
              for G in range([] if dbg == 2 else range(1))[0] if False else (range(0) if dbg == 2 else range(NPIX // GCH)):          # 2 gather chunks of 2304 px
                  rhs_subs = [rhsp.tile([C, NT, SUB], F16, tag=f"rhs{s}",
                                        name=f"rhs{G}_{s}")
                              for s in range(GCH // SUB)]
                  for t in range(NT):
                      q = qpool.tile([C, GCH // 128, 512], F16)
                      src_ap = bass.AP(dup_ap_tensor, 0, [[256, NDUP - 1], [1, 512]])
                      nc.gpsimd.dma_gather(
                          q[:, :, :], src_ap,
                          idx_wt[t][:, G * (GCH // 16):(G + 1) * (GCH // 16)],
                          num_idxs=GCH, num_idxs_reg=GCH, elem_size=512,
                          elem_step=256, single_packet=False)
                      for sub in range(GCH // SUB):  # 6 subchunks of 384 px
                          pv = ps_val.tile([C, SUB], F16)
                          for jj3 in range(3):       # 3 blocks of 128 px
                              jj = sub * 3 + jj3
                              j_global = G * (GCH // 128) + jj
                              wslice = (t, j_global)
                              veng = nc.vector
                              a1 = vpool.tile([128, 128], F16, tag="acc1")
                              veng.tensor_scalar(
                                  out=a1[:, :], in0=q[:, jj, 0:128],
                                  scalar1=w00[:, wslice[0] * NBLK + wslice[1]:
                                              wslice[0] * NBLK + wslice[1] + 1],
                                  scalar2=None, op0=mybir.AluOpType.mult)
                              a2 = vpool.tile([128, 128], F16, tag="acc2")
                              veng.scalar_tensor_tensor(
                                  out=a2[:, :], in0=q[:, jj, 128:256],
                                  scalar=w10[:, wslice[0] * NBLK + wslice[1]:
                                             wslice[0] * NBLK + wslice[1] + 1],
                                  in1=a1[:, :], op0=mybir.AluOpType.mult,
                                  op1=mybir.AluOpType.add)
                              a3 = vpool.tile([128, 128], F16, tag="acc3")
                              veng.scalar_tensor_tensor(
                                  out=a3[:, :], in0=q[:, jj, 256:384],
                                  scalar=w01[:, wslice[0] * NBLK + wslice[1]:
                                             wslice[0] * NBLK + wslice[1] + 1],
                                  in1=a2[:, :], op0=mybir.AluOpType.mult,
                                  op1=mybir.AluOpType.add)
                              val = vpool.tile([128, 128], F16, tag="val")
                              veng.scalar_tensor_tensor(
                                  out=val[:, :], in0=q[:, jj, 384:512],
                                  scalar=w11[:, wslice[0] * NBLK + wslice[1]:
                                             wslice[0] * NBLK + wslice[1] + 1],
                                  in1=a3[:, :], op0=mybir.AluOpType.mult,
                                  op1=mybir.AluOpType.add)
                              nc.tensor.transpose(pv[:, jj3 * 128:(jj3 + 1) * 128],
                                                  val[:, :], ident[:, :])
                          nc.scalar.activation(rhs_subs[sub][:, t, :], pv[:, :],
                                               mybir.ActivationFunctionType.Copy)
                  for sub in range(GCH // SUB):
                      po = ps_out.tile([C, SUB], F32)
                      for t in range(NT):
                          nc.tensor.matmul(po[:, :], w_dc_sb[:, t, :],
                                           rhs_subs[sub][:, t, :],
                                           start=(t == 0), stop=(t == NT - 1))
                      o_sb = outp.tile([C, SUB], F32)
                      nc.scalar.activation(o_sb[:, :], po[:, :],
                                           mybir.ActivationFunctionType.Identity,
                                           bias=dc_b_sb[:, :])
                      nc.sync.dma_start(
                          out.reshape([C, NPIX])[:, G * GCH + sub * SUB:
                                                 G * GCH + (sub + 1) * SUB],
                          o_sb[:, :])

    nc.compile()
    return nc


_NC_CACHE = {}


def _get_nc(dbg=0, reps=1):
    key = f"nc{dbg}_{reps}"
    if key not in _NC_CACHE:
        _NC_CACHE[key] = _build_nc(dbg, reps)
    return _NC_CACHE[key]


def _host_prep(lr_features, hr_features, est_w, est_b, off_w, off_b, dc_w, dc_b):
    """Build the 8 per-core input maps."""
    lr = np.asarray(lr_features, np.float32)
    hr = np.asarray(hr_features, np.float32)

    # weights as lhsT layouts
    w_est = np.transpose(np.asarray(est_w, np.float32), (1, 0, 2, 3)).reshape(
        2, C, 2 * C, 9)
    w_est = np.transpose(w_est, (0, 3, 1, 2)).astype(np.float16)       # [2,9,C,2C]
    w_off = np.transpose(np.asarray(off_w, np.float32), (1, 0, 2, 3)).reshape(
        2, C, 18, 9)
    w_off = np.transpose(w_off, (0, 3, 1, 2)).astype(np.float16)       # [2,9,C,18]
    w_dcT = np.transpose(np.asarray(dc_w, np.float32), (1, 0, 2, 3)).reshape(
        C, C, 9)
    w_dcT = np.transpose(w_dcT, (2, 0, 1)).astype(np.float16)          # [9,C,C]
    est_b2 = np.asarray(est_b, np.float32).reshape(2, C).T.copy()      # [C,2]
    off_b2 = np.asarray(off_b, np.float32).reshape(18, 1)
    dc_b2 = np.asarray(dc_b, np.float32).reshape(C, 1)

    in_maps = []
    for core in range(8):
        b, half = core // 2, core % 2
        h0 = 48 * half
        # conv input slices [C, 52, 98] fp16 (global rows h0-2 .. h0+49)
        def conv_slice(imgt):
            sl = np.zeros((C, HP, WP), np.float16)
            r0, r1 = h0 - 2, h0 + 50
            cr0, cr1 = max(r0, 0), min(r1, H)
            sl[:, cr0 - r0:cr1 - r0, 1:97] = imgt[:, cr0:cr1, :].astype(np.float16)
            return sl
        lr_sl = conv_slice(lr[b])
        hr_sl = conv_slice(hr[b])
        # dup2 pixel-major padded gather image
        pm = np.zeros((PW * PW + 212, C), np.float16)
        pm[:PW * PW] = np.pad(hr[b], ((0, 0), (2, 2), (2, 2))).reshape(
            C, PW * PW).T.astype(np.float16)
        dup = np.zeros((NDUP, 2, C), np.float16)
        dup[:, 0, :] = pm[:NDUP]
        dup[:, 1, :] = pm[100:NDUP + 100]
        # base coords, pixel-major: pixel i = j*128 + p
        i_idx = np.arange(NPIX)
        rows = (h0 + i_idx // 96).astype(np.float32)
        cols = (i_idx % 96).astype(np.float32)
        ky = (np.arange(NT) // 3 - 1).astype(np.float32)
        kx = (np.arange(NT) % 3 - 1).astype(np.float32)
        by = (rows[None, :] + ky[:, None])  # [9, NPIX]
        bx = (cols[None, :] + kx[:, None])
        # [C=128 partitions, 9, 36]: partition p, block j -> pixel j*128+p
        by_t = by.reshape(NT, NBLK, 128).transpose(2, 0, 1).copy()
        bx_t = bx.reshape(NT, NBLK, 128).transpose(2, 0, 1).copy()

        mask = np.broadcast_to(
            np.array([[0.0, 1.0]] if half == 0 else [[1.0, 0.0]], np.float32),
            (C, 2)).copy()
        in_maps.append({
            "lr_pad": lr_sl, "hr_cpad": hr_sl,
            "hr_dup": dup.reshape(-1),
            "w_est": w_est, "w_off": w_off, "w_dc": w_dcT,
            "est_b": est_b2, "off_b": off_b2, "dc_b": dc_b2,
            "base_y": by_t, "base_x": bx_t, "mask_e": mask,
        })
    return in_maps


def kernel(lr_features, hr_features, est_w, est_b, off_w, off_b, dc_w, dc_b):
    nc = _get_nc()
    in_maps = _host_prep(lr_features, hr_features, est_w, est_b,
                         off_w, off_b, dc_w, dc_b)
    res = run_bass_kernel_spmd(nc, in_maps, core_ids=list(range(8))).results
    out = np.empty((B, C, H, W), np.float32)
    for core in range(8):
        b, half = core // 2, core % 2
        h0 = 48 * half
        # out cols: pixel i = j*128+p, i -> (row i//96, col i%96)
        o = res[core]["out"]  # [C, NPIX]
        out[b, :, h0:h0 + 48, :] = o.reshape(C, 48, 96)
    return out



# revision 13
# speedup vs baseline: 1.1395x; 1.1395x over previous
"""Deformable conv block on 8 TRN2 NeuronCores (Bass/Tile) — v2 pipelined.

Sharding: (batch, H-half) -> 8 cores. Each core computes 48 output rows
(all 128 channels) of one batch image.

v2 vs v1: 4-strip software pipeline (12 rows / 1152 px each) so the
PE-bound conv phase overlaps the DVE/Act/Pool/DMA-bound gather phase;
bilinear combine chain split Act->Pool->DVE->DVE; idx DMA path batched
(2+4 DMAs per strip instead of ~20) with an explicit semaphore replacing
the all-engine barrier; deform-conv accumulates per-tap into PSUM.
"""
import sys

sys.path.insert(0, "/opt/trn_rl_repo")

import numpy as np

import concourse.bacc as bacc
import concourse.bass as bass
import concourse.mybir as mybir
from concourse.tile import TileContext
from concourse.masks import make_identity
from concourse.bass_utils import run_bass_kernel_spmd

F32 = mybir.dt.float32
F16 = mybir.dt.float16
I16 = mybir.dt.int16

B, C, H, W = 4, 128, 96, 96
NPIX = 48 * 96                 # pixels per core (half image)
HP, WP = 52, 98                # conv input slice: 48+2*2 rows, 96+2 cols
PW = 100                       # padded gather image width/height
NDUP = 10112                   # dup2 rows (>= 10001, mult of 128)
NT = 9
NBLK = NPIX // 128             # 36 pixel blocks of 128
NS = 4                         # strips
RS = 12                        # rows per strip
PS = RS * 96                   # 1152 px per strip
JB = PS // 128                 # 9 pixel blocks per strip
MAGIC = float(3 * 2 ** 22)     # RNE-integer rounding constant
MUL = mybir.AluOpType.mult
ADD = mybir.AluOpType.add


def _build_nc(dbg=0, reps=1):
    nc = bacc.Bacc("TRN2", target_bir_lowering=False, debug=False)

    lr_pad = nc.dram_tensor("lr_pad", [C, HP, WP], F16, kind="ExternalInput")
    hr_cpad = nc.dram_tensor("hr_cpad", [C, HP, WP], F16, kind="ExternalInput")
    hr_dup = nc.dram_tensor("hr_dup", [NDUP * 256], F16, kind="ExternalInput")
    w_est = nc.dram_tensor("w_est", [2, NT, C, 2 * C], F16, kind="ExternalInput")
    w_off = nc.dram_tensor("w_off", [2, NT, C, 18], F16, kind="ExternalInput")
    w_dc = nc.dram_tensor("w_dc", [NT, C, C], F16, kind="ExternalInput")
    est_b = nc.dram_tensor("est_b", [C, 2], F32, kind="ExternalInput")
    off_b = nc.dram_tensor("off_b", [18, 1], F32, kind="ExternalInput")
    dc_b = nc.dram_tensor("dc_b", [C, 1], F32, kind="ExternalInput")
    base_y = nc.dram_tensor("base_y", [C, NT, NBLK], F32, kind="ExternalInput")
    base_x = nc.dram_tensor("base_x", [C, NT, NBLK], F32, kind="ExternalInput")
    mask_e = nc.dram_tensor("mask_e", [C, 2], F32, kind="ExternalInput")

    out = nc.dram_tensor("out", [C, NPIX], F32, kind="ExternalOutput")
    idx_dram = nc.dram_tensor("idx_dram", [NT * NPIX], I16)

    with TileContext(nc) as tc:
        with tc.tile_pool(name="wsb", bufs=1) as wsb, \
             tc.tile_pool(name="img", bufs=1) as img, \
             tc.tile_pool(name="estp", bufs=1) as estp, \
             tc.tile_pool(name="offp", bufs=2) as offp, \
             tc.tile_pool(name="math", bufs=2) as mathp, \
             tc.tile_pool(name="idxp", bufs=2) as idxp, \
             tc.tile_pool(name="iwp", bufs=4) as iwp, \
             tc.tile_pool(name="qpool", bufs=2) as qpool, \
             tc.tile_pool(name="vpool", bufs=4) as vpool, \
             tc.tile_pool(name="valp", bufs=12) as valp, \
             tc.tile_pool(name="rhsp", bufs=2) as rhsp, \
             tc.tile_pool(name="outp", bufs=2) as outp, \
             tc.tile_pool(name="ps_conv", bufs=2, space="PSUM") as ps_conv, \
             tc.tile_pool(name="ps_off", bufs=1, space="PSUM") as ps_off, \
             tc.tile_pool(name="ps_val", bufs=1, space="PSUM") as ps_val, \
             tc.tile_pool(name="ps_out", bufs=1, space="PSUM") as ps_out:

            # ---------------- loads (SP-issued) ----------------
            lr_sb = img.tile([C, HP * WP], F16)
            nc.sync.dma_start(lr_sb, lr_pad.reshape([C, HP * WP])[:, :])
            hr_sb = img.tile([C, HP * WP], F16)
            nc.sync.dma_start(hr_sb, hr_cpad.reshape([C, HP * WP])[:, :])
            w_est_sb = wsb.tile([C, 2, NT, 2 * C], F16)
            nc.sync.dma_start(
                w_est_sb,
                bass.AP(w_est.reshape([2 * NT * C * 2 * C])[:].tensor, 0,
                        [[2 * C, C], [NT * C * 2 * C, 2], [C * 2 * C, NT],
                         [1, 2 * C]]))
            w_off_sb = wsb.tile([C, 2, NT, 18], F16)
            nc.sync.dma_start(
                w_off_sb,
                bass.AP(w_off.reshape([2 * NT * C * 18])[:].tensor, 0,
                        [[18, C], [NT * C * 18, 2], [C * 18, NT], [1, 18]]))
            w_dc_sb = wsb.tile([C, NT, C], F16)
            nc.sync.dma_start(
                w_dc_sb,
                bass.AP(w_dc.reshape([NT * C * C])[:].tensor, 0,
                        [[C, C], [C * C, NT], [1, C]]))
            est_b_sb = wsb.tile([C, 2], F32)
            nc.sync.dma_start(est_b_sb, est_b[:, :])
            off_b_sb = wsb.tile([18, 1], F32)
            nc.sync.dma_start(off_b_sb, off_b[:, :])
            dc_b_sb = wsb.tile([C, 1], F32)
            nc.sync.dma_start(dc_b_sb, dc_b[:, :])
            by_sb = wsb.tile([C, NT, NBLK], F32)
            nc.sync.dma_start(by_sb, base_y[:, :, :])
            bx_sb = wsb.tile([C, NT, NBLK], F32)
            nc.sync.dma_start(bx_sb, base_x[:, :, :])
            mask_sb = wsb.tile([C, 2], F32)
            nc.sync.dma_start(mask_sb, mask_e[:, :])
            ident = wsb.tile([128, 128], F16)
            make_identity(nc, ident)
            ident32 = wsb.tile([32, 32], F32)
            make_identity(nc, ident32)
            r_nidx3 = nc.gpsimd.to_reg(3 * PS)

            est_s = {}      # (s, cb) -> [C, 14*98] f16, slice rows 12s+1..+14
            off_s = {}      # s -> [18, PS] f32
            w4_s = {}       # s -> dict of w00..w11 [C, NT*JB] f32
            iw_s = {}       # s -> idx_wt tile [128, NT*72] i16
            val_s = {}      # (s, t) -> [128, JB*128] f16 pixel-major combined
            dup_ap_tensor = hr_dup[:].tensor

            # ------------- per-strip program pieces -------------
            def conv_front(s, math_eng=None):
                """conv1 + conv2 + off transposes + math + idx DMAs, strip s."""
                ve = math_eng or nc.vector
                r_lo = 12 * s + 1          # first est slice row of strip tile
                for cb in range(2):
                    t_ = estp.tile([C, 14 * WP], F16, tag=f"est{s}_{cb}",
                                   name=f"est{s}_{cb}")
                    est_s[(s, cb)] = t_
                    nc.gpsimd.memset(
                        bass.AP(t_.tensor, t_.offset, [t_.ap[0], [WP, 14], [1, 1]]),
                        0.0)
                    nc.gpsimd.memset(
                        bass.AP(t_.tensor, t_.offset + WP - 1,
                                [t_.ap[0], [WP, 14], [1, 1]]), 0.0)
                # conv1: est rows r_lo..r_lo+13 in groups of 5,5,4
                for cb in range(2):
                    dst = est_s[(s, cb)]
                    for g, (g0, nr) in enumerate(((0, 5), (5, 5), (10, 4))):
                        r0 = r_lo + g0
                        ps = ps_conv.tile([C, nr * 96], F32, tag="c1")
                        n = 0
                        for cin in range(2):
                            src = lr_sb if cin == 0 else hr_sb
                            for t in range(NT):
                                ky, kx = t // 3 - 1, t % 3 - 1
                                rhs = bass.AP(src.tensor,
                                              src.offset + (r0 + ky) * WP + kx + 1,
                                              [src.ap[0], [WP, nr], [1, 96]])
                                lhsT = w_est_sb[:, cin, t, cb * C:(cb + 1) * C]
                                nc.tensor.matmul(ps[:, :], lhsT, rhs,
                                                 start=(n == 0), stop=(n == 17))
                                n += 1
                        dsta = bass.AP(dst.tensor, dst.offset + g0 * WP + 1,
                                       [dst.ap[0], [WP, nr], [1, 96]])
                        nc.scalar.activation(dsta, ps[:, :],
                                             mybir.ActivationFunctionType.Relu,
                                             bias=est_b_sb[:, cb:cb + 1])
                    # mask out-of-image halo rows (slice row 1 / 50)
                    if s == 0:
                        ve.tensor_scalar(
                            out=dst[:, 0:WP], in0=dst[:, 0:WP],
                            scalar1=mask_sb[:, 0:1], scalar2=None, op0=MUL)
                    if s == NS - 1:
                        ve.tensor_scalar(
                            out=dst[:, 13 * WP:14 * WP],
                            in0=dst[:, 13 * WP:14 * WP],
                            scalar1=mask_sb[:, 1:2], scalar2=None, op0=MUL)

                # conv2: offsets for out rows 12s..12s+11, groups of 4
                off_t = offp.tile([18, PS], F32, tag="off", name=f"off{s}")
                off_s[s] = off_t
                for g in range(3):
                    ps = ps_off.tile([18, 384], F32, tag="c2", name=f"c2_{s}_{g}")
                    n = 0
                    for cin in range(2):
                        src = est_s[(s, cin)]
                        for t in range(NT):
                            ky, kx = t // 3 - 1, t % 3 - 1
                            rhs = bass.AP(src.tensor,
                                          src.offset + (4 * g + 1 + ky) * WP + kx + 1,
                                          [src.ap[0], [WP, 4], [1, 96]])
                            nc.tensor.matmul(ps[:, :], w_off_sb[:, cin, t, :], rhs,
                                             start=(n == 0), stop=(n == 17))
                            n += 1
                    nc.scalar.activation(off_t[:, g * 384:(g + 1) * 384], ps[:, :],
                                         mybir.ActivationFunctionType.Identity,
                                         bias=off_b_sb[:, :])

                # off -> pixel-major via PE transposes
                TAPS = NT * JB  # 81
                off_pm = mathp.tile([C, JB, 18], F32, tag="offpm", name=f"offpm{s}")
                for j in range(JB):
                    pst = ps_off.tile([C, 18], F32, tag="c2", name=f"pot{s}_{j}")
                    nc.tensor.transpose(pst[:, :], off_t[:, j * 128:(j + 1) * 128],
                                        ident32[0:18, 0:18])
                    nc.scalar.activation(off_pm[:, j, :], pst[:, :],
                                         mybir.ActivationFunctionType.Copy)
                dy = bass.AP(off_pm.tensor, off_pm.offset,
                             [off_pm.ap[0], [2, NT], [18, JB]])
                dx = bass.AP(off_pm.tensor, off_pm.offset + 1,
                             [off_pm.ap[0], [2, NT], [18, JB]])

                def mt(tag):
                    return mathp.tile([C, TAPS], F32, tag=tag, name=f"{tag}{s}")

                def floor_frac(v, ax):
                    t_ = mt("ff_t" + ax)
                    ve.tensor_scalar(out=t_[:, :], in0=v[:, :], scalar1=MAGIC,
                                            scalar2=MAGIC, op0=ADD,
                                            op1=mybir.AluOpType.subtract)
                    g_ = mt("ff_g" + ax)
                    ve.tensor_tensor(out=g_[:, :], in0=t_[:, :], in1=v[:, :],
                                            op=mybir.AluOpType.is_gt)
                    fl = mt("ff_fl" + ax)
                    ve.tensor_tensor(out=fl[:, :], in0=t_[:, :], in1=g_[:, :],
                                            op=mybir.AluOpType.subtract)
                    fr = mt("ff_fr" + ax)
                    ve.tensor_tensor(out=fr[:, :], in0=v[:, :], in1=fl[:, :],
                                            op=mybir.AluOpType.subtract)
                    return fl, fr

                by_ap = bass.AP(by_sb.tensor, by_sb.offset + s * JB,
                                [by_sb.ap[0], [NBLK, NT], [1, JB]])
                bx_ap = bass.AP(bx_sb.tensor, bx_sb.offset + s * JB,
                                [bx_sb.ap[0], [NBLK, NT], [1, JB]])
                py = mt("py")
                ve.tensor_tensor(out=py[:, :], in0=dy[:, :], in1=by_ap,
                                        op=ADD)
                ve.tensor_scalar(out=py[:, :], in0=py[:, :], scalar1=-2.0,
                                        scalar2=96.0, op0=mybir.AluOpType.max,
                                        op1=mybir.AluOpType.min)
                px = mt("px")
                ve.tensor_tensor(out=px[:, :], in0=dx[:, :], in1=bx_ap,
                                        op=ADD)
                ve.tensor_scalar(out=px[:, :], in0=px[:, :], scalar1=-2.0,
                                        scalar2=96.0, op0=mybir.AluOpType.max,
                                        op1=mybir.AluOpType.min)
                y0, ly = floor_frac(py, "y")
                x0, lx = floor_frac(px, "x")

                wy0 = mt("wy0")
                ve.tensor_scalar(out=wy0[:, :], in0=ly[:, :], scalar1=-1.0,
                                        scalar2=1.0, op0=MUL, op1=ADD)
                wx0 = mt("wx0")
                ve.tensor_scalar(out=wx0[:, :], in0=lx[:, :], scalar1=-1.0,
                                        scalar2=1.0, op0=MUL, op1=ADD)
                w00 = mt("w00")
                ve.tensor_tensor(out=w00[:, :], in0=wy0[:, :], in1=wx0[:, :],
                                        op=MUL)
                w10 = mt("w10")
                ve.tensor_tensor(out=w10[:, :], in0=ly[:, :], in1=wx0[:, :],
                                        op=MUL)
                w01 = mt("w01")
                ve.tensor_tensor(out=w01[:, :], in0=wy0[:, :], in1=lx[:, :],
                                        op=MUL)
                w11 = mt("w11")
                ve.tensor_tensor(out=w11[:, :], in0=ly[:, :], in1=lx[:, :],
                                        op=MUL)
                w4_s[s] = {"w00": w00, "w10": w10, "w01": w01, "w11": w11}

                # gather index: (y0+2)*100 + (x0+2) = 100*y0 + x0 + 202
                idxf = mt("idxf")
                ve.scalar_tensor_tensor(out=idxf[:, :], in0=y0[:, :],
                                               scalar=100.0, in1=x0[:, :],
                                               op0=MUL, op1=ADD)
                ve.tensor_scalar(out=idxf[:, :], in0=idxf[:, :],
                                        scalar1=202.0, scalar2=None, op0=ADD)
                idx16 = idxp.tile([C, TAPS], I16, tag="idx16", name=f"idx16_{s}")
                ve.tensor_copy(idx16[:, :], idxf[:, :])
                # one DMA: idx16 -> dram at D = s*NT*PS + t*PS + j*128 + p
                nc.sync.dma_start(
                    bass.AP(idx_dram[:].tensor, s * NT * PS,
                            [[1, 128], [PS, NT], [128, JB]]),
                    idx16[:, :])

            def idx_fetch(s):
                """Wrapped idx read + replicate to 128 partitions (SP)."""
                iw = iwp.tile([128, NT * 72], I16, tag="iw", name=f"iw{s}")
                iw_s[s] = iw
                nc.sync.dma_start(
                    bass.AP(iw.tensor, iw.offset,
                            [[iw.ap[0][0], 16], [72, NT], [1, 72]]),
                    bass.AP(idx_dram[:].tensor, s * NT * PS,
                            [[1, 16], [PS, NT], [16, 72]]))
                for gsz in (16, 32, 64):
                    nc.sync.dma_start(iw[gsz:2 * gsz, :], iw[0:gsz, :])

            def gather_combine(s):
                """3 three-tap gathers + bilinear combine, strip s."""
                w4 = w4_s[s]
                for tg in range(3):
                    q = qpool.tile([C, 3 * JB, 512], F16, tag="q")
                    src_ap = bass.AP(dup_ap_tensor, 0, [[256, NDUP - 1], [1, 512]])
                    nc.gpsimd.dma_gather(
                        q[:, :, :], src_ap,
                        iw_s[s][:, tg * 3 * 72:(tg + 1) * 3 * 72],
                        num_idxs=3 * PS, num_idxs_reg=r_nidx3, elem_size=512,
                        elem_step=256, single_packet=False)
                    for dt_ in range(3):
                        t = tg * 3 + dt_
                        val = valp.tile([128, JB * 128], F16, tag="val",
                                        name=f"val{s}_{t}")
                        val_s[(s, t)] = val
                        for j in range(JB):
                            col = t * JB + j
                            qj = dt_ * JB + j
                            h1 = vpool.tile([128, 128], F16, tag="h1")
                            nc.scalar.activation(
                                h1[:, :], q[:, qj, 0:128],
                                mybir.ActivationFunctionType.Identity,
                                bias=0.0, scale=w4["w00"][:, col:col + 1])
                            if t % 2 == 0:
                                # W2: Pool takes corners q2 (mult) + merge
                                p1 = vpool.tile([128, 128], F16, tag="p1")
                                nc.gpsimd.tensor_scalar(
                                    out=p1[:, :], in0=q[:, qj, 256:384],
                                    scalar1=w4["w01"][:, col:col + 1],
                                    scalar2=None, op0=MUL)
                                p2 = vpool.tile([128, 128], F16, tag="p2")
                                nc.gpsimd.tensor_tensor(
                                    out=p2[:, :], in0=p1[:, :], in1=h1[:, :],
                                    op=ADD)
                                d1 = vpool.tile([128, 128], F16, tag="d1")
                                nc.vector.scalar_tensor_tensor(
                                    out=d1[:, :], in0=q[:, qj, 128:256],
                                    scalar=w4["w10"][:, col:col + 1], in1=p2[:, :],
                                    op0=MUL, op1=ADD)
                            else:
                                # W1: all-DVE chain after the Act mult
                                d0 = vpool.tile([128, 128], F16, tag="d0")
                                nc.vector.scalar_tensor_tensor(
                                    out=d0[:, :], in0=q[:, qj, 128:256],
                                    scalar=w4["w10"][:, col:col + 1], in1=h1[:, :],
                                    op0=MUL, op1=ADD)
                                d1 = vpool.tile([128, 128], F16, tag="d1")
                                nc.vector.scalar_tensor_tensor(
                                    out=d1[:, :], in0=q[:, qj, 256:384],
                                    scalar=w4["w01"][:, col:col + 1], in1=d0[:, :],
                                    op0=MUL, op1=ADD)
                            nc.vector.scalar_tensor_tensor(
                                out=val[:, j * 128:(j + 1) * 128],
                                in0=q[:, qj, 384:512],
                                scalar=w4["w11"][:, col:col + 1], in1=d1[:, :],
                                op0=MUL, op1=ADD)

            ps_out_tiles = {}

            def tail(s, t):
                """PE transposes + PSUM->SBUF copy + dc matmul, strip s tap t."""
                val = val_s.pop((s, t))
                pv = ps_val.tile([C, JB * 128], F16, tag="pv")
                for j in range(JB):
                    nc.tensor.matmul(pv[:, j * 128:(j + 1) * 128],
                                     val[:, j * 128:(j + 1) * 128], ident[:, :],
                                     is_transpose=True, start=True, stop=True)
                rhs = rhsp.tile([C, PS], F16, tag="rhs")
                nc.scalar.activation(rhs[:, :], pv[:, :],
                                     mybir.ActivationFunctionType.Copy)
                if t == 0:
                    ps_out_tiles[s] = [
                        ps_out.tile([C, 384], F32, tag=f"po{si}",
                                    name=f"po{s}_{si}")
                        for si in range(3)]
                for si in range(3):
                    nc.tensor.matmul(ps_out_tiles[s][si][:, :], w_dc_sb[:, t, :],
                                     rhs[:, si * 384:(si + 1) * 384],
                                     start=(t == 0), stop=(t == NT - 1))

            def strip_out(s):
                o_sb = outp.tile([C, PS], F32, tag="osb")
                for si in range(3):
                    nc.scalar.activation(o_sb[:, si * 384:(si + 1) * 384],
                                         ps_out_tiles[s][si][:, :],
                                         mybir.ActivationFunctionType.Identity,
                                         bias=dc_b_sb[:, :])
                nc.scalar.dma_start(
                    out.reshape([C, NPIX])[:, s * PS:(s + 1) * PS], o_sb[:, :])

            # ---------------- emission schedule ----------------
            # In-order sequencers: keep each engine's stream free of
            # early-strip work that depends on late-strip producers.
            conv_front(0)
            conv_front(1)
            idx_fetch(0)
            gather_combine(0)
            conv_front(2)
            idx_fetch(1)
            gather_combine(1)
            for t in range(NT):
                tail(0, t)
            strip_out(0)
            idx_fetch(2)
            gather_combine(2)
            conv_front(3)
            for t in range(NT):
                tail(1, t)
            strip_out(1)
            idx_fetch(3)
            gather_combine(3)
            for t in range(NT):
                tail(2, t)
            strip_out(2)
            for t in range(NT):
                tail(3, t)
            strip_out(3)

    nc.compile()
    return nc


_NC_CACHE = {}


def _get_nc(dbg=0, reps=1):
    key = f"nc{dbg}_{reps}"
    if key not in _NC_CACHE:
        _NC_CACHE[key] = _build_nc(dbg, reps)
    return _NC_CACHE[key]


def _host_prep(lr_features, hr_features, est_w, est_b, off_w, off_b, dc_w, dc_b):
    """Build the 8 per-core input maps."""
    lr = np.asarray(lr_features, np.float32)
    hr = np.asarray(hr_features, np.float32)

    # weights as lhsT layouts
    w_est = np.transpose(np.asarray(est_w, np.float32), (1, 0, 2, 3)).reshape(
        2, C, 2 * C, 9)
    w_est = np.transpose(w_est, (0, 3, 1, 2)).astype(np.float16)       # [2,9,C,2C]
    w_off = np.transpose(np.asarray(off_w, np.float32), (1, 0, 2, 3)).reshape(
        2, C, 18, 9)
    w_off = np.transpose(w_off, (0, 3, 1, 2)).astype(np.float16)       # [2,9,C,18]
    w_dcT = np.transpose(np.asarray(dc_w, np.float32), (1, 0, 2, 3)).reshape(
        C, C, 9)
    w_dcT = np.transpose(w_dcT, (2, 0, 1)).astype(np.float16)          # [9,C,C]
    est_b2 = np.asarray(est_b, np.float32).reshape(2, C).T.copy()      # [C,2]
    off_b2 = np.asarray(off_b, np.float32).reshape(18, 1)
    dc_b2 = np.asarray(dc_b, np.float32).reshape(C, 1)

    in_maps = []
    for core in range(8):
        b, half = core // 2, core % 2
        h0 = 48 * half
        # conv input slices [C, 52, 98] fp16 (global rows h0-2 .. h0+49)
        def conv_slice(imgt):
            sl = np.zeros((C, HP, WP), np.float16)
            r0, r1 = h0 - 2, h0 + 50
            cr0, cr1 = max(r0, 0), min(r1, H)
            sl[:, cr0 - r0:cr1 - r0, 1:97] = imgt[:, cr0:cr1, :].astype(np.float16)
            return sl
        lr_sl = conv_slice(lr[b])
        hr_sl = conv_slice(hr[b])
        # dup2 pixel-major padded gather image
        pm = np.zeros((PW * PW + 212, C), np.float16)
        pm[:PW * PW] = np.pad(hr[b], ((0, 0), (2, 2), (2, 2))).reshape(
            C, PW * PW).T.astype(np.float16)
        dup = np.zeros((NDUP, 2, C), np.float16)
        dup[:, 0, :] = pm[:NDUP]
        dup[:, 1, :] = pm[100:NDUP + 100]
        # base coords, pixel-major: pixel i = j*128 + p
        i_idx = np.arange(NPIX)
        rows = (h0 + i_idx // 96).astype(np.float32)
        cols = (i_idx % 96).astype(np.float32)
        ky = (np.arange(NT) // 3 - 1).astype(np.float32)
        kx = (np.arange(NT) % 3 - 1).astype(np.float32)
        by = (rows[None, :] + ky[:, None])  # [9, NPIX]
        bx = (cols[None, :] + kx[:, None])
        # [C=128 partitions, 9, 36]: partition p, block j -> pixel j*128+p
        by_t = by.reshape(NT, NBLK, 128).transpose(2, 0, 1).copy()
        bx_t = bx.reshape(NT, NBLK, 128).transpose(2, 0, 1).copy()

        mask = np.broadcast_to(
            np.array([[0.0, 1.0]] if half == 0 else [[1.0, 0.0]], np.float32),
            (C, 2)).copy()
        in_maps.append({
            "lr_pad": lr_sl, "hr_cpad": hr_sl,
            "hr_dup": dup.reshape(-1),
            "w_est": w_est, "w_off": w_off, "w_dc": w_dcT,
            "est_b": est_b2, "off_b": off_b2, "dc_b": dc_b2,
            "base_y": by_t, "base_x": bx_t, "mask_e": mask,
        })
    return in_maps


def kernel(lr_features, hr_features, est_w, est_b, off_w, off_b, dc_w, dc_b):
    nc = _get_nc()
    in_maps = _host_prep(lr_features, hr_features, est_w, est_b,
                         off_w, off_b, dc_w, dc_b)
    res = run_bass_kernel_spmd(nc, in_maps, core_ids=list(range(8))).results
    out = np.empty((B, C, H, W), np.float32)
    for core in range(8):
        b, half = core // 2, core % 2
        h0 = 48 * half
        o = res[core]["out"]  # [C, NPIX]
        out[b, :, h0:h0 + 48, :] = o.reshape(C, 48, 96)
    return out


# revision 29
# speedup vs baseline: 1.2627x; 1.1082x over previous
"""Deformable conv block on 8 TRN2 NeuronCores (Bass/Tile) — v2 pipelined.

Sharding: (batch, H-half) -> 8 cores. Each core computes 48 output rows
(all 128 channels) of one batch image.

v2 vs v1: 4-strip software pipeline (12 rows / 1152 px each) so the
PE-bound conv phase overlaps the DVE/Act/Pool/DMA-bound gather phase;
bilinear combine chain split Act->Pool->DVE->DVE; idx DMA path batched
(2+4 DMAs per strip instead of ~20) with an explicit semaphore replacing
the all-engine barrier; deform-conv accumulates per-tap into PSUM.
"""
import sys

sys.path.insert(0, "/opt/trn_rl_repo")

import numpy as np

import concourse.bacc as bacc
import concourse.bass as bass
import concourse.mybir as mybir
from concourse.tile import TileContext
from concourse.masks import make_identity
from concourse.bass_utils import run_bass_kernel_spmd

F32 = mybir.dt.float32
F16 = mybir.dt.float16
I16 = mybir.dt.int16

B, C, H, W = 4, 128, 96, 96
NPIX = 48 * 96                 # pixels per core (half image)
HP, WP = 52, 98                # conv input slice: 48+2*2 rows, 96+2 cols
PW = 100                       # padded gather image width/height
NDUP = 10112                   # dup2 rows (>= 10001, mult of 128)
NT = 9
NBLK = NPIX // 128             # 36 pixel blocks of 128
NS = 4                         # strips
RS = 12                        # rows per strip
PS = RS * 96                   # 1152 px per strip
JB = PS // 128                 # 9 pixel blocks per strip
MAGIC = float(3 * 2 ** 22)     # RNE-integer rounding constant
MUL = mybir.AluOpType.mult
ADD = mybir.AluOpType.add


def _build_nc(dbg=0, reps=1):
    nc = bacc.Bacc("TRN2", target_bir_lowering=False, debug=False)

    lr_pad = nc.dram_tensor("lr_pad", [C, HP, WP], F16, kind="ExternalInput")
    hr_cpad = nc.dram_tensor("hr_cpad", [C, HP, WP], F16, kind="ExternalInput")
    hr_dup = nc.dram_tensor("hr_dup", [NDUP * 256], F16, kind="ExternalInput")
    w_est = nc.dram_tensor("w_est", [2, NT, C, 2 * C], F16, kind="ExternalInput")
    w_off = nc.dram_tensor("w_off", [2, NT, C, 18], F16, kind="ExternalInput")
    w_dc = nc.dram_tensor("w_dc", [NT, C, C], F16, kind="ExternalInput")
    est_b = nc.dram_tensor("est_b", [C, 2], F32, kind="ExternalInput")
    off_b = nc.dram_tensor("off_b", [18, 1], F32, kind="ExternalInput")
    dc_b = nc.dram_tensor("dc_b", [C, 1], F32, kind="ExternalInput")
    base_y = nc.dram_tensor("base_y", [C, NT, NBLK], F32, kind="ExternalInput")
    base_x = nc.dram_tensor("base_x", [C, NT, NBLK], F32, kind="ExternalInput")
    mask_e = nc.dram_tensor("mask_e", [C, 2], F32, kind="ExternalInput")

    out = nc.dram_tensor("out", [C, NPIX], F16, kind="ExternalOutput")
    idx_dram = nc.dram_tensor("idx_dram", [NT * NPIX], I16)

    with TileContext(nc) as tc:
        with tc.tile_pool(name="wsb", bufs=1) as wsb, \
             tc.tile_pool(name="img", bufs=1) as img, \
             tc.tile_pool(name="estp", bufs=1) as estp, \
             tc.tile_pool(name="offp", bufs=2) as offp, \
             tc.tile_pool(name="math", bufs=2) as mathp, \
             tc.tile_pool(name="idxp", bufs=2) as idxp, \
             tc.tile_pool(name="iwp", bufs=4) as iwp, \
             tc.tile_pool(name="qpool", bufs=3) as qpool, \
             tc.tile_pool(name="vpool", bufs=4) as vpool, \
             tc.tile_pool(name="valp", bufs=12) as valp, \
             tc.tile_pool(name="rhsp", bufs=2) as rhsp, \
             tc.tile_pool(name="outp", bufs=2) as outp, \
             tc.tile_pool(name="ps_conv", bufs=2, space="PSUM") as ps_conv, \
             tc.tile_pool(name="ps_off", bufs=1, space="PSUM") as ps_off, \
             tc.tile_pool(name="ps_val", bufs=1, space="PSUM") as ps_val, \
             tc.tile_pool(name="ps_out", bufs=1, space="PSUM") as ps_out:

            # ---------------- loads (SP-issued) ----------------
            lr_sb = img.tile([C, HP * WP], F16)
            nc.sync.dma_start(lr_sb, lr_pad.reshape([C, HP * WP])[:, :])
            hr_sb = img.tile([C, HP * WP], F16)
            nc.sync.dma_start(hr_sb, hr_cpad.reshape([C, HP * WP])[:, :])
            w_est_sb = wsb.tile([C, 2, NT, 2 * C], F16)
            nc.sync.dma_start(
                w_est_sb,
                bass.AP(w_est.reshape([2 * NT * C * 2 * C])[:].tensor, 0,
                        [[2 * C, C], [NT * C * 2 * C, 2], [C * 2 * C, NT],
                         [1, 2 * C]]))
            w_off_sb = wsb.tile([C, 2, NT, 18], F16)
            nc.sync.dma_start(
                w_off_sb,
                bass.AP(w_off.reshape([2 * NT * C * 18])[:].tensor, 0,
                        [[18, C], [NT * C * 18, 2], [C * 18, NT], [1, 18]]))
            w_dc_sb = wsb.tile([C, NT, C], F16)
            nc.sync.dma_start(
                w_dc_sb,
                bass.AP(w_dc.reshape([NT * C * C])[:].tensor, 0,
                        [[C, C], [C * C, NT], [1, C]]))
            est_b_sb = wsb.tile([C, 2], F32)
            nc.sync.dma_start(est_b_sb, est_b[:, :])
            off_b_sb = wsb.tile([18, 1], F32)
            nc.sync.dma_start(off_b_sb, off_b[:, :])
            dc_b_sb = wsb.tile([C, 1], F32)
            nc.sync.dma_start(dc_b_sb, dc_b[:, :])
            by_sb = wsb.tile([C, NT, NBLK], F32)
            nc.sync.dma_start(by_sb, base_y[:, :, :])
            bx_sb = wsb.tile([C, NT, NBLK], F32)
            nc.sync.dma_start(bx_sb, base_x[:, :, :])
            mask_sb = wsb.tile([C, 2], F32)
            nc.sync.dma_start(mask_sb, mask_e[:, :])
            ident = wsb.tile([128, 128], F16)
            make_identity(nc, ident)
            ident32 = wsb.tile([32, 32], F32)
            make_identity(nc, ident32)
            r_nidx2 = nc.gpsimd.to_reg(2 * PS)
            r_nidx1 = nc.gpsimd.to_reg(PS)

            est_s = {}      # (s, cb) -> [C, 14*98] f16, slice rows 12s+1..+14
            off_s = {}      # s -> [18, PS] f32
            w4_s = {}       # s -> dict of w00..w11 [C, NT*JB] f32
            iw_s = {}       # s -> idx_wt tile [128, NT*72] i16
            val_s = {}      # (s, t) -> [128, JB*128] f16 pixel-major combined
            dup_ap_tensor = hr_dup[:].tensor

            # ------------- per-strip program pieces -------------
            def cf_chunks(s):
                """9 emit-callables: 6 conv1 (cb,g) chunks + 3 conv2 groups."""
                r_lo = 12 * s + 1
                for cb in range(2):
                    t_ = estp.tile([C, 14 * WP], F16, tag=f"est{s}_{cb}",
                                   name=f"est{s}_{cb}")
                    est_s[(s, cb)] = t_
                    nc.gpsimd.memset(
                        bass.AP(t_.tensor, t_.offset, [t_.ap[0], [WP, 14], [1, 1]]),
                        0.0)
                    nc.gpsimd.memset(
                        bass.AP(t_.tensor, t_.offset + WP - 1,
                                [t_.ap[0], [WP, 14], [1, 1]]), 0.0)

                def conv1_chunk(cb, g):
                    g0, nr = ((0, 5), (5, 5), (10, 4))[g]
                    dst = est_s[(s, cb)]
                    r0 = r_lo + g0
                    ps = ps_conv.tile([C, nr * 96], F32, tag="c1",
                                      name=f"c1_{s}_{cb}_{g}")
                    n = 0
                    for cin in range(2):
                        src_ = lr_sb if cin == 0 else hr_sb
                        for t in range(NT):
                            ky, kx = t // 3 - 1, t % 3 - 1
                            rhs = bass.AP(src_.tensor,
                                          src_.offset + (r0 + ky) * WP + kx + 1,
                                          [src_.ap[0], [WP, nr], [1, 96]])
                            lhsT = w_est_sb[:, cin, t, cb * C:(cb + 1) * C]
                            nc.tensor.matmul(ps[:, :], lhsT, rhs,
                                             start=(n == 0), stop=(n == 17))
                            n += 1
                    dsta = bass.AP(dst.tensor, dst.offset + g0 * WP + 1,
                                   [dst.ap[0], [WP, nr], [1, 96]])
                    nc.scalar.activation(dsta, ps[:, :],
                                         mybir.ActivationFunctionType.Relu,
                                         bias=est_b_sb[:, cb:cb + 1])
                    # mask out-of-image halo rows (slice row 1 / 50)
                    if s == 0 and g == 0:
                        nc.vector.tensor_scalar(
                            out=dst[:, 0:WP], in0=dst[:, 0:WP],
                            scalar1=mask_sb[:, 0:1], scalar2=None, op0=MUL)
                    if s == NS - 1 and g == 2:
                        nc.vector.tensor_scalar(
                            out=dst[:, 13 * WP:14 * WP],
                            in0=dst[:, 13 * WP:14 * WP],
                            scalar1=mask_sb[:, 1:2], scalar2=None, op0=MUL)

                def conv2_chunk(g):
                    if g == 0:
                        off_s[s] = offp.tile([18, PS], F32, tag="off",
                                             name=f"off{s}")
                    off_t = off_s[s]
                    ps = ps_off.tile([18, 384], F32, tag="c2", name=f"c2_{s}_{g}")
                    n = 0
                    for cin in range(2):
                        src_ = est_s[(s, cin)]
                        for t in range(NT):
                            ky, kx = t // 3 - 1, t % 3 - 1
                            rhs = bass.AP(src_.tensor,
                                          src_.offset + (4 * g + 1 + ky) * WP + kx + 1,
                                          [src_.ap[0], [WP, 4], [1, 96]])
                            nc.tensor.matmul(ps[:, :], w_off_sb[:, cin, t, :], rhs,
                                             start=(n == 0), stop=(n == 17))
                            n += 1
                    nc.scalar.activation(off_t[:, g * 384:(g + 1) * 384], ps[:, :],
                                         mybir.ActivationFunctionType.Identity,
                                         bias=off_b_sb[:, :])

                return [lambda cb=cb, g=g: conv1_chunk(cb, g)
                        for g in range(3) for cb in range(2)][:4] +                        [lambda: conv2_chunk(0),
                        lambda: conv1_chunk(0, 2), lambda: conv1_chunk(1, 2),
                        lambda: conv2_chunk(1), lambda: conv2_chunk(2)]

            def cf_math(s):
                """off transposes + idx/weight math + idx write DMA, strip s."""
                ve = nc.vector
                off_t = off_s[s]
                TAPS = NT * JB  # 81
                off_pm = mathp.tile([C, JB, 18], F32, tag="offpm", name=f"offpm{s}")
                pst = ps_off.tile([C, JB * 18], F32, tag="c2", name=f"pot{s}")
                for j in range(JB):
                    nc.tensor.transpose(pst[:, j * 18:(j + 1) * 18],
                                        off_t[:, j * 128:(j + 1) * 128],
                                        ident32[0:18, 0:18])
                nc.vector.tensor_copy(off_pm[:, :, :], pst[:, :])
                dy = bass.AP(off_pm.tensor, off_pm.offset,
                             [off_pm.ap[0], [2, NT], [18, JB]])
                dx = bass.AP(off_pm.tensor, off_pm.offset + 1,
                             [off_pm.ap[0], [2, NT], [18, JB]])

                def mt(tag):
                    return mathp.tile([C, TAPS], F32, tag=tag, name=f"{tag}{s}")

                def floor_frac(v, ax):
                    t_ = mt("ff_t" + ax)
                    ve.tensor_scalar(out=t_[:, :], in0=v[:, :], scalar1=MAGIC,
                                     scalar2=MAGIC, op0=ADD,
                                     op1=mybir.AluOpType.subtract)
                    g_ = mt("ff_g" + ax)
                    ve.tensor_tensor(out=g_[:, :], in0=t_[:, :], in1=v[:, :],
                                     op=mybir.AluOpType.is_gt)
                    fl = mt("ff_fl" + ax)
                    ve.tensor_tensor(out=fl[:, :], in0=t_[:, :], in1=g_[:, :],
                                     op=mybir.AluOpType.subtract)
                    fr = mt("ff_fr" + ax)
                    ve.tensor_tensor(out=fr[:, :], in0=v[:, :], in1=fl[:, :],
                                     op=mybir.AluOpType.subtract)
                    return fl, fr

                by_ap = bass.AP(by_sb.tensor, by_sb.offset + s * JB,
                                [by_sb.ap[0], [NBLK, NT], [1, JB]])
                bx_ap = bass.AP(bx_sb.tensor, bx_sb.offset + s * JB,
                                [bx_sb.ap[0], [NBLK, NT], [1, JB]])
                py = mt("py")
                ve.tensor_tensor(out=py[:, :], in0=dy[:, :], in1=by_ap, op=ADD)
                ve.tensor_scalar(out=py[:, :], in0=py[:, :], scalar1=-2.0,
                                 scalar2=96.0, op0=mybir.AluOpType.max,
                                 op1=mybir.AluOpType.min)
                px = mt("px")
                ve.tensor_tensor(out=px[:, :], in0=dx[:, :], in1=bx_ap, op=ADD)
                ve.tensor_scalar(out=px[:, :], in0=px[:, :], scalar1=-2.0,
                                 scalar2=96.0, op0=mybir.AluOpType.max,
                                 op1=mybir.AluOpType.min)
                y0, ly = floor_frac(py, "y")
                x0, lx = floor_frac(px, "x")

                wy0 = mt("wy0")
                ve.tensor_scalar(out=wy0[:, :], in0=ly[:, :], scalar1=-1.0,
                                 scalar2=1.0, op0=MUL, op1=ADD)
                wx0 = mt("wx0")
                ve.tensor_scalar(out=wx0[:, :], in0=lx[:, :], scalar1=-1.0,
                                 scalar2=1.0, op0=MUL, op1=ADD)
                w00 = mt("w00")
                ve.tensor_tensor(out=w00[:, :], in0=wy0[:, :], in1=wx0[:, :],
                                 op=MUL)
                w10 = mt("w10")
                ve.tensor_tensor(out=w10[:, :], in0=ly[:, :], in1=wx0[:, :],
                                 op=MUL)
                w01 = mt("w01")
                ve.tensor_tensor(out=w01[:, :], in0=wy0[:, :], in1=lx[:, :],
                                 op=MUL)
                w11 = mt("w11")
                ve.tensor_tensor(out=w11[:, :], in0=ly[:, :], in1=lx[:, :],
                                 op=MUL)
                w4_s[s] = {"w00": w00, "w10": w10, "w01": w01, "w11": w11}

                idxf = mt("idxf")
                ve.scalar_tensor_tensor(out=idxf[:, :], in0=y0[:, :],
                                        scalar=100.0, in1=x0[:, :],
                                        op0=MUL, op1=ADD)
                ve.tensor_scalar(out=idxf[:, :], in0=idxf[:, :],
                                 scalar1=202.0, scalar2=None, op0=ADD)
                idx16 = idxp.tile([C, TAPS], I16, tag="idx16", name=f"idx16_{s}")
                ve.tensor_copy(idx16[:, :], idxf[:, :])
                nc.sync.dma_start(
                    bass.AP(idx_dram[:].tensor, s * NT * PS,
                            [[1, 128], [PS, NT], [128, JB]]),
                    idx16[:, :])

            def idx_fetch(s):
                """Wrapped idx read + replicate to 128 partitions (SP)."""
                iw = iwp.tile([128, NT * 72], I16, tag="iw", name=f"iw{s}")
                iw_s[s] = iw
                nc.sync.dma_start(
                    bass.AP(iw.tensor, iw.offset,
                            [[iw.ap[0][0], 16], [72, NT], [1, 72]]),
                    bass.AP(idx_dram[:].tensor, s * NT * PS,
                            [[1, 16], [PS, NT], [16, 72]]))
                for gsz in (16, 32, 64):
                    nc.sync.dma_start(iw[gsz:2 * gsz, :], iw[0:gsz, :])

            TG = [(0, 2), (2, 2), (4, 2), (6, 2), (8, 1)]

            def gather_taps(s, tg):
                """One gather of TG[tg] taps, strip s."""
                t0, nt = TG[tg]
                q = qpool.tile([C, 2 * JB, 512], F16, tag="q", name=f"q{s}_{tg}")
                q_s[(s, tg)] = q
                src_ap = bass.AP(dup_ap_tensor, 0, [[256, NDUP - 1], [1, 512]])
                nc.gpsimd.dma_gather(
                    bass.AP(q.tensor, q.offset,
                            [q.ap[0], [512, nt * JB], [1, 512]]), src_ap,
                    iw_s[s][:, t0 * 72:(t0 + nt) * 72],
                    num_idxs=nt * PS, num_idxs_reg=(r_nidx2 if nt == 2 else r_nidx1),
                    elem_size=512, elem_step=256, single_packet=False)

            def combine_tap(s, t):
                """Bilinear combine for one tap (9 blocks)."""
                w4 = w4_s[s]
                q = q_s[(s, t // 2)]
                dt_ = t % 2
                val = valp.tile([128, JB * 128], F16, tag="val",
                                name=f"val{s}_{t}")
                val_s[(s, t)] = val
                for j in range(JB):
                    col = t * JB + j
                    qj = dt_ * JB + j
                    v = col % 9
                    if v == 5 or v == 8:
                        h1 = vpool.tile([128, 128], F16, tag="h1")
                        nc.vector.tensor_scalar(
                            out=h1[:, :], in0=q[:, qj, 0:128],
                            scalar1=w4["w00"][:, col:col + 1], scalar2=None,
                            op0=MUL)
                    else:
                        h1 = vpool.tile([128, 128], F16, tag="h1")
                        nc.scalar.activation(
                            h1[:, :], q[:, qj, 0:128],
                            mybir.ActivationFunctionType.Identity,
                            bias=0.0, scale=w4["w00"][:, col:col + 1])
                    if v >= 6:
                        # W1a: all-DVE after the Act mult
                        d0 = vpool.tile([128, 128], F16, tag="d0")
                        nc.vector.scalar_tensor_tensor(
                            out=d0[:, :], in0=q[:, qj, 256:384],
                            scalar=w4["w01"][:, col:col + 1], in1=h1[:, :],
                            op0=MUL, op1=ADD)
                        d1 = vpool.tile([128, 128], F16, tag="d1")
                        nc.vector.scalar_tensor_tensor(
                            out=d1[:, :], in0=q[:, qj, 128:256],
                            scalar=w4["w10"][:, col:col + 1], in1=d0[:, :],
                            op0=MUL, op1=ADD)
                    else:
                        # Vc: Pool takes corner q2 (mult) + merge with h1
                        p1 = vpool.tile([128, 128], F16, tag="p1")
                        nc.gpsimd.tensor_scalar(
                            out=p1[:, :], in0=q[:, qj, 256:384],
                            scalar1=w4["w01"][:, col:col + 1],
                            scalar2=None, op0=MUL)
                        p2 = vpool.tile([128, 128], F16, tag="p2")
                        nc.gpsimd.tensor_tensor(
                            out=p2[:, :], in0=p1[:, :], in1=h1[:, :], op=ADD)
                        d1 = vpool.tile([128, 128], F16, tag="d1")
                        nc.vector.scalar_tensor_tensor(
                            out=d1[:, :], in0=q[:, qj, 128:256],
                            scalar=w4["w10"][:, col:col + 1], in1=p2[:, :],
                            op0=MUL, op1=ADD)
                    nc.vector.scalar_tensor_tensor(
                        out=val[:, j * 128:(j + 1) * 128],
                        in0=q[:, qj, 384:512],
                        scalar=w4["w11"][:, col:col + 1], in1=d1[:, :],
                        op0=MUL, op1=ADD)

            ps_out_tiles = {}

            def tail(s, t):
                """PE transposes + PSUM->SBUF copy + dc matmul, strip s tap t."""
                val = val_s.pop((s, t))
                pv = ps_val.tile([C, JB * 128], F16, tag="pv")
                for j in range(JB):
                    nc.tensor.matmul(pv[:, j * 128:(j + 1) * 128],
                                     val[:, j * 128:(j + 1) * 128], ident[:, :],
                                     is_transpose=True, start=True, stop=True)
                rhs = rhsp.tile([C, PS], F16, tag="rhs")
                nc.scalar.activation(rhs[:, :], pv[:, :],
                                     mybir.ActivationFunctionType.Copy)
                if t == 0:
                    ps_out_tiles[s] = [
                        ps_out.tile([C, 384], F32, tag=f"po{si}",
                                    name=f"po{s}_{si}")
                        for si in range(3)]
                for si in range(3):
                    nc.tensor.matmul(ps_out_tiles[s][si][:, :], w_dc_sb[:, t, :],
                                     rhs[:, si * 384:(si + 1) * 384],
                                     start=(t == 0), stop=(t == NT - 1))

            def strip_out(s):
                o_sb = outp.tile([C, PS], F16, tag="osb")
                for si in range(3):
                    nc.scalar.activation(o_sb[:, si * 384:(si + 1) * 384],
                                         ps_out_tiles[s][si][:, :],
                                         mybir.ActivationFunctionType.Identity,
                                         bias=dc_b_sb[:, :])
                nc.scalar.dma_start(
                    out.reshape([C, NPIX])[:, s * PS:(s + 1) * PS], o_sb[:, :])

            # ---------------- emission schedule ----------------
            # Tap-granularity interleave: conv chunks of strip s+1 between
            # combine taps of strip s so no engine stream has long bursts
            # blocking another engine's PSUM drain.
            q_s = {}
            for fn in cf_chunks(0):
                fn()
            cf_math(0)
            idx_fetch(0)
            for s in range(NS):
                parts = cf_chunks(s + 1) if s + 1 < NS else [lambda: None] * 9
                gsched = {0: 0, 2: 1, 4: 2, 6: 3, 8: 4}
                for k in range(NT):
                    if k in gsched:
                        gather_taps(s, gsched[k])
                    parts[k]()
                    combine_tap(s, k)
                    if k >= 3:
                        tail(s, k - 3)
                if s + 1 < NS:
                    cf_math(s + 1)
                    idx_fetch(s + 1)
                for k in range(NT - 3, NT):
                    tail(s, k)
                strip_out(s)

    nc.compile()
    return nc


_NC_CACHE = {}


def _get_nc(dbg=0, reps=1):
    key = f"nc{dbg}_{reps}"
    if key not in _NC_CACHE:
        _NC_CACHE[key] = _build_nc(dbg, reps)
    return _NC_CACHE[key]


def _host_prep(lr_features, hr_features, est_w, est_b, off_w, off_b, dc_w, dc_b):
    """Build the 8 per-core input maps."""
    lr = np.asarray(lr_features, np.float32)
    hr = np.asarray(hr_features, np.float32)

    # weights as lhsT layouts
    w_est = np.transpose(np.asarray(est_w, np.float32), (1, 0, 2, 3)).reshape(
        2, C, 2 * C, 9)
    w_est = np.transpose(w_est, (0, 3, 1, 2)).astype(np.float16)       # [2,9,C,2C]
    w_off = np.transpose(np.asarray(off_w, np.float32), (1, 0, 2, 3)).reshape(
        2, C, 18, 9)
    w_off = np.transpose(w_off, (0, 3, 1, 2)).astype(np.float16)       # [2,9,C,18]
    w_dcT = np.transpose(np.asarray(dc_w, np.float32), (1, 0, 2, 3)).reshape(
        C, C, 9)
    w_dcT = np.transpose(w_dcT, (2, 0, 1)).astype(np.float16)          # [9,C,C]
    est_b2 = np.asarray(est_b, np.float32).reshape(2, C).T.copy()      # [C,2]
    off_b2 = np.asarray(off_b, np.float32).reshape(18, 1)
    dc_b2 = np.asarray(dc_b, np.float32).reshape(C, 1)

    in_maps = []
    for core in range(8):
        b, half = core // 2, core % 2
        h0 = 48 * half
        # conv input slices [C, 52, 98] fp16 (global rows h0-2 .. h0+49)
        def conv_slice(imgt):
            sl = np.zeros((C, HP, WP), np.float16)
            r0, r1 = h0 - 2, h0 + 50
            cr0, cr1 = max(r0, 0), min(r1, H)
            sl[:, cr0 - r0:cr1 - r0, 1:97] = imgt[:, cr0:cr1, :].astype(np.float16)
            return sl
        lr_sl = conv_slice(lr[b])
        hr_sl = conv_slice(hr[b])
        # dup2 pixel-major padded gather image
        pm = np.zeros((PW * PW + 212, C), np.float16)
        pm[:PW * PW] = np.pad(hr[b], ((0, 0), (2, 2), (2, 2))).reshape(
            C, PW * PW).T.astype(np.float16)
        dup = np.zeros((NDUP, 2, C), np.float16)
        dup[:, 0, :] = pm[:NDUP]
        dup[:, 1, :] = pm[100:NDUP + 100]
        # base coords, pixel-major: pixel i = j*128 + p
        i_idx = np.arange(NPIX)
        rows = (h0 + i_idx // 96).astype(np.float32)
        cols = (i_idx % 96).astype(np.float32)
        ky = (np.arange(NT) // 3 - 1).astype(np.float32)
        kx = (np.arange(NT) % 3 - 1).astype(np.float32)
        by = (rows[None, :] + ky[:, None])  # [9, NPIX]
        bx = (cols[None, :] + kx[:, None])
        # [C=128 partitions, 9, 36]: partition p, block j -> pixel j*128+p
        by_t = by.reshape(NT, NBLK, 128).transpose(2, 0, 1).copy()
        bx_t = bx.reshape(NT, NBLK, 128).transpose(2, 0, 1).copy()

        mask = np.broadcast_to(
            np.array([[0.0, 1.0]] if half == 0 else [[1.0, 0.0]], np.float32),
            (C, 2)).copy()
        in_maps.append({
            "lr_pad": lr_sl, "hr_cpad": hr_sl,
            "hr_dup": dup.reshape(-1),
            "w_est": w_est, "w_off": w_off, "w_dc": w_dcT,
            "est_b": est_b2, "off_b": off_b2, "dc_b": dc_b2,
            "base_y": by_t, "base_x": bx_t, "mask_e": mask,
        })
    return in_maps


def kernel(lr_features, hr_features, est_w, est_b, off_w, off_b, dc_w, dc_b):
    nc = _get_nc()
    in_maps = _host_prep(lr_features, hr_features, est_w, est_b,
                         off_w, off_b, dc_w, dc_b)
    res = run_bass_kernel_spmd(nc, in_maps, core_ids=list(range(8))).results
    out = np.empty((B, C, H, W), np.float32)
    for core in range(8):
        b, half = core // 2, core % 2
        h0 = 48 * half
        o = res[core]["out"]  # [C, NPIX]
        out[b, :, h0:h0 + 48, :] = o.reshape(C, 48, 96)
    return out


# revision 30
# speedup vs baseline: 1.2869x; 1.0192x over previous
"""Deformable conv block on 8 TRN2 NeuronCores (Bass/Tile) — v2 pipelined.

Sharding: (batch, H-half) -> 8 cores. Each core computes 48 output rows
(all 128 channels) of one batch image.

v2 vs v1: 4-strip software pipeline (12 rows / 1152 px each) so the
PE-bound conv phase overlaps the DVE/Act/Pool/DMA-bound gather phase;
bilinear combine chain split Act->Pool->DVE->DVE; idx DMA path batched
(2+4 DMAs per strip instead of ~20) with an explicit semaphore replacing
the all-engine barrier; deform-conv accumulates per-tap into PSUM.
"""
import sys

sys.path.insert(0, "/opt/trn_rl_repo")

import numpy as np

import concourse.bacc as bacc
import concourse.bass as bass
import concourse.mybir as mybir
from concourse.tile import TileContext
from concourse.masks import make_identity
from concourse.bass_utils import run_bass_kernel_spmd

F32 = mybir.dt.float32
F16 = mybir.dt.float16
I16 = mybir.dt.int16

B, C, H, W = 4, 128, 96, 96
NPIX = 48 * 96                 # pixels per core (half image)
HP, WP = 52, 98                # conv input slice: 48+2*2 rows, 96+2 cols
PW = 100                       # padded gather image width/height
NDUP = 10112                   # dup2 rows (>= 10001, mult of 128)
NT = 9
NBLK = NPIX // 128             # 36 pixel blocks of 128
NS = 4                         # strips
RS = 12                        # rows per strip
PS = RS * 96                   # 1152 px per strip
JB = PS // 128                 # 9 pixel blocks per strip
MAGIC = float(3 * 2 ** 22)     # RNE-integer rounding constant
MUL = mybir.AluOpType.mult
ADD = mybir.AluOpType.add


def _build_nc(dbg=0, reps=1):
    nc = bacc.Bacc("TRN2", target_bir_lowering=False, debug=False)

    lr_pad = nc.dram_tensor("lr_pad", [C, HP, WP], F16, kind="ExternalInput")
    hr_cpad = nc.dram_tensor("hr_cpad", [C, HP, WP], F16, kind="ExternalInput")
    hr_dup = nc.dram_tensor("hr_dup", [NDUP * 256], F16, kind="ExternalInput")
    w_est = nc.dram_tensor("w_est", [2, NT, C, 2 * C], F16, kind="ExternalInput")
    w_off = nc.dram_tensor("w_off", [2, NT, C, 18], F16, kind="ExternalInput")
    w_dc = nc.dram_tensor("w_dc", [NT, C, C], F16, kind="ExternalInput")
    est_b = nc.dram_tensor("est_b", [C, 2], F32, kind="ExternalInput")
    off_b = nc.dram_tensor("off_b", [18, 1], F32, kind="ExternalInput")
    dc_b = nc.dram_tensor("dc_b", [C, 1], F32, kind="ExternalInput")
    base_y = nc.dram_tensor("base_y", [C, NT, NBLK], F32, kind="ExternalInput")
    base_x = nc.dram_tensor("base_x", [C, NT, NBLK], F32, kind="ExternalInput")
    mask_e = nc.dram_tensor("mask_e", [C, 2], F32, kind="ExternalInput")

    out = nc.dram_tensor("out", [C, NPIX], F16, kind="ExternalOutput")
    idx_dram = nc.dram_tensor("idx_dram", [NT * NPIX], I16)

    with TileContext(nc) as tc:
        with tc.tile_pool(name="wsb", bufs=1) as wsb, \
             tc.tile_pool(name="img", bufs=1) as img, \
             tc.tile_pool(name="estp", bufs=1) as estp, \
             tc.tile_pool(name="offp", bufs=2) as offp, \
             tc.tile_pool(name="math", bufs=2) as mathp, \
             tc.tile_pool(name="idxp", bufs=2) as idxp, \
             tc.tile_pool(name="iwp", bufs=4) as iwp, \
             tc.tile_pool(name="qpool", bufs=3) as qpool, \
             tc.tile_pool(name="vpool", bufs=4) as vpool, \
             tc.tile_pool(name="valp", bufs=12) as valp, \
             tc.tile_pool(name="rhsp", bufs=2) as rhsp, \
             tc.tile_pool(name="outp", bufs=2) as outp, \
             tc.tile_pool(name="ps_conv", bufs=2, space="PSUM") as ps_conv, \
             tc.tile_pool(name="ps_off", bufs=1, space="PSUM") as ps_off, \
             tc.tile_pool(name="ps_val", bufs=1, space="PSUM") as ps_val, \
             tc.tile_pool(name="ps_out", bufs=1, space="PSUM") as ps_out:

            # ---------------- loads (SP-issued) ----------------
            lr_sb = img.tile([C, HP * WP], F16)
            nc.sync.dma_start(lr_sb, lr_pad.reshape([C, HP * WP])[:, :])
            w_est_sb = wsb.tile([C, 2, NT, 2 * C], F16)
            nc.sync.dma_start(
                w_est_sb[:, 0, :, :],
                bass.AP(w_est.reshape([2 * NT * C * 2 * C])[:].tensor, 0,
                        [[2 * C, C], [C * 2 * C, NT], [1, 2 * C]]))
            hr_sb = img.tile([C, HP * WP], F16)
            nc.sync.dma_start(hr_sb, hr_cpad.reshape([C, HP * WP])[:, :])
            nc.sync.dma_start(
                w_est_sb[:, 1, :, :],
                bass.AP(w_est.reshape([2 * NT * C * 2 * C])[:].tensor,
                        NT * C * 2 * C,
                        [[2 * C, C], [C * 2 * C, NT], [1, 2 * C]]))
            w_off_sb = wsb.tile([C, 2, NT, 18], F16)
            nc.sync.dma_start(
                w_off_sb,
                bass.AP(w_off.reshape([2 * NT * C * 18])[:].tensor, 0,
                        [[18, C], [NT * C * 18, 2], [C * 18, NT], [1, 18]]))
            w_dc_sb = wsb.tile([C, NT, C], F16)
            nc.sync.dma_start(
                w_dc_sb,
                bass.AP(w_dc.reshape([NT * C * C])[:].tensor, 0,
                        [[C, C], [C * C, NT], [1, C]]))
            est_b_sb = wsb.tile([C, 2], F32)
            nc.sync.dma_start(est_b_sb, est_b[:, :])
            off_b_sb = wsb.tile([18, 1], F32)
            nc.sync.dma_start(off_b_sb, off_b[:, :])
            dc_b_sb = wsb.tile([C, 1], F32)
            nc.sync.dma_start(dc_b_sb, dc_b[:, :])
            by_sb = wsb.tile([C, NT, NBLK], F32)
            nc.sync.dma_start(by_sb, base_y[:, :, :])
            bx_sb = wsb.tile([C, NT, NBLK], F32)
            nc.sync.dma_start(bx_sb, base_x[:, :, :])
            mask_sb = wsb.tile([C, 2], F32)
            nc.sync.dma_start(mask_sb, mask_e[:, :])
            ident = wsb.tile([128, 128], F16)
            make_identity(nc, ident)
            ident32 = wsb.tile([32, 32], F32)
            make_identity(nc, ident32)
            r_nidx2 = nc.gpsimd.to_reg(2 * PS)
            r_nidx1 = nc.gpsimd.to_reg(PS)

            est_s = {}      # (s, cb) -> [C, 14*98] f16, slice rows 12s+1..+14
            off_s = {}      # s -> [18, PS] f32
            w4_s = {}       # s -> dict of w00..w11 [C, NT*JB] f32
            iw_s = {}       # s -> idx_wt tile [128, NT*72] i16
            val_s = {}      # (s, t) -> [128, JB*128] f16 pixel-major combined
            dup_ap_tensor = hr_dup[:].tensor

            # ------------- per-strip program pieces -------------
            def cf_chunks(s):
                """9 emit-callables: 6 conv1 (cb,g) chunks + 3 conv2 groups."""
                r_lo = 12 * s + 1
                for cb in range(2):
                    t_ = estp.tile([C, 14 * WP], F16, tag=f"est{s}_{cb}",
                                   name=f"est{s}_{cb}")
                    est_s[(s, cb)] = t_
                    nc.gpsimd.memset(
                        bass.AP(t_.tensor, t_.offset, [t_.ap[0], [WP, 14], [1, 1]]),
                        0.0)
                    nc.gpsimd.memset(
                        bass.AP(t_.tensor, t_.offset + WP - 1,
                                [t_.ap[0], [WP, 14], [1, 1]]), 0.0)

                def conv1_chunk(cb, g):
                    g0, nr = ((0, 5), (5, 5), (10, 4))[g]
                    dst = est_s[(s, cb)]
                    r0 = r_lo + g0
                    ps = ps_conv.tile([C, nr * 96], F32, tag="c1",
                                      name=f"c1_{s}_{cb}_{g}")
                    n = 0
                    for cin in range(2):
                        src_ = lr_sb if cin == 0 else hr_sb
                        for t in range(NT):
                            ky, kx = t // 3 - 1, t % 3 - 1
                            rhs = bass.AP(src_.tensor,
                                          src_.offset + (r0 + ky) * WP + kx + 1,
                                          [src_.ap[0], [WP, nr], [1, 96]])
                            lhsT = w_est_sb[:, cin, t, cb * C:(cb + 1) * C]
                            nc.tensor.matmul(ps[:, :], lhsT, rhs,
                                             start=(n == 0), stop=(n == 17))
                            n += 1
                    dsta = bass.AP(dst.tensor, dst.offset + g0 * WP + 1,
                                   [dst.ap[0], [WP, nr], [1, 96]])
                    nc.scalar.activation(dsta, ps[:, :],
                                         mybir.ActivationFunctionType.Relu,
                                         bias=est_b_sb[:, cb:cb + 1])
                    # mask out-of-image halo rows (slice row 1 / 50)
                    if s == 0 and g == 0:
                        nc.vector.tensor_scalar(
                            out=dst[:, 0:WP], in0=dst[:, 0:WP],
                            scalar1=mask_sb[:, 0:1], scalar2=None, op0=MUL)
                    if s == NS - 1 and g == 2:
                        nc.vector.tensor_scalar(
                            out=dst[:, 13 * WP:14 * WP],
                            in0=dst[:, 13 * WP:14 * WP],
                            scalar1=mask_sb[:, 1:2], scalar2=None, op0=MUL)

                def conv2_chunk(g):
                    if g == 0:
                        off_s[s] = offp.tile([18, PS], F32, tag="off",
                                             name=f"off{s}")
                    off_t = off_s[s]
                    ps = ps_off.tile([18, 384], F32, tag="c2", name=f"c2_{s}_{g}")
                    n = 0
                    for cin in range(2):
                        src_ = est_s[(s, cin)]
                        for t in range(NT):
                            ky, kx = t // 3 - 1, t % 3 - 1
                            rhs = bass.AP(src_.tensor,
                                          src_.offset + (4 * g + 1 + ky) * WP + kx + 1,
                                          [src_.ap[0], [WP, 4], [1, 96]])
                            nc.tensor.matmul(ps[:, :], w_off_sb[:, cin, t, :], rhs,
                                             start=(n == 0), stop=(n == 17))
                            n += 1
                    nc.scalar.activation(off_t[:, g * 384:(g + 1) * 384], ps[:, :],
                                         mybir.ActivationFunctionType.Identity,
                                         bias=off_b_sb[:, :])

                return [lambda cb=cb, g=g: conv1_chunk(cb, g)
                        for g in range(3) for cb in range(2)][:4] +                        [lambda: conv2_chunk(0),
                        lambda: conv1_chunk(0, 2), lambda: conv1_chunk(1, 2),
                        lambda: conv2_chunk(1), lambda: conv2_chunk(2)]

            def cf_math(s):
                """off transposes + idx/weight math + idx write DMA, strip s."""
                ve = nc.vector
                off_t = off_s[s]
                TAPS = NT * JB  # 81
                off_pm = mathp.tile([C, JB, 18], F32, tag="offpm", name=f"offpm{s}")
                pst = ps_off.tile([C, JB * 18], F32, tag="c2", name=f"pot{s}")
                for j in range(JB):
                    nc.tensor.transpose(pst[:, j * 18:(j + 1) * 18],
                                        off_t[:, j * 128:(j + 1) * 128],
                                        ident32[0:18, 0:18])
                nc.vector.tensor_copy(off_pm[:, :, :], pst[:, :])
                dy = bass.AP(off_pm.tensor, off_pm.offset,
                             [off_pm.ap[0], [2, NT], [18, JB]])
                dx = bass.AP(off_pm.tensor, off_pm.offset + 1,
                             [off_pm.ap[0], [2, NT], [18, JB]])

                def mt(tag):
                    return mathp.tile([C, TAPS], F32, tag=tag, name=f"{tag}{s}")

                def floor_frac(v, ax):
                    t_ = mt("ff_t" + ax)
                    ve.tensor_scalar(out=t_[:, :], in0=v[:, :], scalar1=MAGIC,
                                     scalar2=MAGIC, op0=ADD,
                                     op1=mybir.AluOpType.subtract)
                    g_ = mt("ff_g" + ax)
                    ve.tensor_tensor(out=g_[:, :], in0=t_[:, :], in1=v[:, :],
                                     op=mybir.AluOpType.is_gt)
                    fl = mt("ff_fl" + ax)
                    ve.tensor_tensor(out=fl[:, :], in0=t_[:, :], in1=g_[:, :],
                                     op=mybir.AluOpType.subtract)
                    fr = mt("ff_fr" + ax)
                    ve.tensor_tensor(out=fr[:, :], in0=v[:, :], in1=fl[:, :],
                                     op=mybir.AluOpType.subtract)
                    return fl, fr

                by_ap = bass.AP(by_sb.tensor, by_sb.offset + s * JB,
                                [by_sb.ap[0], [NBLK, NT], [1, JB]])
                bx_ap = bass.AP(bx_sb.tensor, bx_sb.offset + s * JB,
                                [bx_sb.ap[0], [NBLK, NT], [1, JB]])
                py = mt("py")
                ve.tensor_tensor(out=py[:, :], in0=dy[:, :], in1=by_ap, op=ADD)
                ve.tensor_scalar(out=py[:, :], in0=py[:, :], scalar1=-2.0,
                                 scalar2=96.0, op0=mybir.AluOpType.max,
                                 op1=mybir.AluOpType.min)
                px = mt("px")
                ve.tensor_tensor(out=px[:, :], in0=dx[:, :], in1=bx_ap, op=ADD)
                ve.tensor_scalar(out=px[:, :], in0=px[:, :], scalar1=-2.0,
                                 scalar2=96.0, op0=mybir.AluOpType.max,
                                 op1=mybir.AluOpType.min)
                y0, ly = floor_frac(py, "y")
                x0, lx = floor_frac(px, "x")

                wy0 = mt("wy0")
                ve.tensor_scalar(out=wy0[:, :], in0=ly[:, :], scalar1=-1.0,
                                 scalar2=1.0, op0=MUL, op1=ADD)
                wx0 = mt("wx0")
                ve.tensor_scalar(out=wx0[:, :], in0=lx[:, :], scalar1=-1.0,
                                 scalar2=1.0, op0=MUL, op1=ADD)
                w00 = mt("w00")
                ve.tensor_tensor(out=w00[:, :], in0=wy0[:, :], in1=wx0[:, :],
                                 op=MUL)
                w10 = mt("w10")
                ve.tensor_tensor(out=w10[:, :], in0=ly[:, :], in1=wx0[:, :],
                                 op=MUL)
                w01 = mt("w01")
                ve.tensor_tensor(out=w01[:, :], in0=wy0[:, :], in1=lx[:, :],
                                 op=MUL)
                w11 = mt("w11")
                ve.tensor_tensor(out=w11[:, :], in0=ly[:, :], in1=lx[:, :],
                                 op=MUL)
                w4_s[s] = {"w00": w00, "w10": w10, "w01": w01, "w11": w11}

                idxf = mt("idxf")
                ve.scalar_tensor_tensor(out=idxf[:, :], in0=y0[:, :],
                                        scalar=100.0, in1=x0[:, :],
                                        op0=MUL, op1=ADD)
                ve.tensor_scalar(out=idxf[:, :], in0=idxf[:, :],
                                 scalar1=202.0, scalar2=None, op0=ADD)
                idx16 = idxp.tile([C, TAPS], I16, tag="idx16", name=f"idx16_{s}")
                ve.tensor_copy(idx16[:, :], idxf[:, :])
                nc.sync.dma_start(
                    bass.AP(idx_dram[:].tensor, s * NT * PS,
                            [[1, 128], [PS, NT], [128, JB]]),
                    idx16[:, :])

            def idx_fetch(s):
                """Wrapped idx read + replicate to 128 partitions (SP)."""
                iw = iwp.tile([128, NT * 72], I16, tag="iw", name=f"iw{s}")
                iw_s[s] = iw
                nc.sync.dma_start(
                    bass.AP(iw.tensor, iw.offset,
                            [[iw.ap[0][0], 16], [72, NT], [1, 72]]),
                    bass.AP(idx_dram[:].tensor, s * NT * PS,
                            [[1, 16], [PS, NT], [16, 72]]))
                for gsz in (16, 32, 64):
                    nc.sync.dma_start(iw[gsz:2 * gsz, :], iw[0:gsz, :])

            TG = [(0, 2), (2, 2), (4, 2), (6, 2), (8, 1)]

            def gather_taps(s, tg):
                """One gather of TG[tg] taps, strip s."""
                t0, nt = TG[tg]
                q = qpool.tile([C, 2 * JB, 512], F16, tag="q", name=f"q{s}_{tg}")
                q_s[(s, tg)] = q
                src_ap = bass.AP(dup_ap_tensor, 0, [[256, NDUP - 1], [1, 512]])
                nc.gpsimd.dma_gather(
                    bass.AP(q.tensor, q.offset,
                            [q.ap[0], [512, nt * JB], [1, 512]]), src_ap,
                    iw_s[s][:, t0 * 72:(t0 + nt) * 72],
                    num_idxs=nt * PS, num_idxs_reg=(r_nidx2 if nt == 2 else r_nidx1),
                    elem_size=512, elem_step=256, single_packet=False)

            def combine_tap(s, t):
                """Bilinear combine for one tap (9 blocks)."""
                w4 = w4_s[s]
                q = q_s[(s, t // 2)]
                dt_ = t % 2
                val = valp.tile([128, JB * 128], F16, tag="val",
                                name=f"val{s}_{t}")
                val_s[(s, t)] = val
                for j in range(JB):
                    col = t * JB + j
                    qj = dt_ * JB + j
                    v = col % 9
                    if v == 5 or v == 8:
                        h1 = vpool.tile([128, 128], F16, tag="h1")
                        nc.vector.tensor_scalar(
                            out=h1[:, :], in0=q[:, qj, 0:128],
                            scalar1=w4["w00"][:, col:col + 1], scalar2=None,
                            op0=MUL)
                    else:
                        h1 = vpool.tile([128, 128], F16, tag="h1")
                        nc.scalar.activation(
                            h1[:, :], q[:, qj, 0:128],
                            mybir.ActivationFunctionType.Identity,
                            bias=0.0, scale=w4["w00"][:, col:col + 1])
                    if v >= 6:
                        # W1a: all-DVE after the Act mult
                        d0 = vpool.tile([128, 128], F16, tag="d0")
                        nc.vector.scalar_tensor_tensor(
                            out=d0[:, :], in0=q[:, qj, 256:384],
                            scalar=w4["w01"][:, col:col + 1], in1=h1[:, :],
                            op0=MUL, op1=ADD)
                        d1 = vpool.tile([128, 128], F16, tag="d1")
                        nc.vector.scalar_tensor_tensor(
                            out=d1[:, :], in0=q[:, qj, 128:256],
                            scalar=w4["w10"][:, col:col + 1], in1=d0[:, :],
                            op0=MUL, op1=ADD)
                    else:
                        # Vc: Pool takes corner q2 (mult) + merge with h1
                        p1 = vpool.tile([128, 128], F16, tag="p1")
                        nc.gpsimd.tensor_scalar(
                            out=p1[:, :], in0=q[:, qj, 256:384],
                            scalar1=w4["w01"][:, col:col + 1],
                            scalar2=None, op0=MUL)
                        p2 = vpool.tile([128, 128], F16, tag="p2")
                        nc.gpsimd.tensor_tensor(
                            out=p2[:, :], in0=p1[:, :], in1=h1[:, :], op=ADD)
                        d1 = vpool.tile([128, 128], F16, tag="d1")
                        nc.vector.scalar_tensor_tensor(
                            out=d1[:, :], in0=q[:, qj, 128:256],
                            scalar=w4["w10"][:, col:col + 1], in1=p2[:, :],
                            op0=MUL, op1=ADD)
                    nc.vector.scalar_tensor_tensor(
                        out=val[:, j * 128:(j + 1) * 128],
                        in0=q[:, qj, 384:512],
                        scalar=w4["w11"][:, col:col + 1], in1=d1[:, :],
                        op0=MUL, op1=ADD)

            ps_out_tiles = {}

            def tail(s, t):
                """PE transposes + PSUM->SBUF copy + dc matmul, strip s tap t."""
                val = val_s.pop((s, t))
                pv = ps_val.tile([C, JB * 128], F16, tag="pv")
                for j in range(JB):
                    nc.tensor.matmul(pv[:, j * 128:(j + 1) * 128],
                                     val[:, j * 128:(j + 1) * 128], ident[:, :],
                                     is_transpose=True, start=True, stop=True)
                rhs = rhsp.tile([C, PS], F16, tag="rhs")
                nc.scalar.activation(rhs[:, :], pv[:, :],
                                     mybir.ActivationFunctionType.Copy)
                if t == 0:
                    ps_out_tiles[s] = [
                        ps_out.tile([C, 384], F32, tag=f"po{si}",
                                    name=f"po{s}_{si}")
                        for si in range(3)]
                for si in range(3):
                    nc.tensor.matmul(ps_out_tiles[s][si][:, :], w_dc_sb[:, t, :],
                                     rhs[:, si * 384:(si + 1) * 384],
                                     start=(t == 0), stop=(t == NT - 1))

            def strip_out(s):
                o_sb = outp.tile([C, PS], F16, tag="osb")
                for si in range(3):
                    nc.scalar.activation(o_sb[:, si * 384:(si + 1) * 384],
                                         ps_out_tiles[s][si][:, :],
                                         mybir.ActivationFunctionType.Identity,
                                         bias=dc_b_sb[:, :])
                nc.scalar.dma_start(
                    out.reshape([C, NPIX])[:, s * PS:(s + 1) * PS], o_sb[:, :])

            # ---------------- emission schedule ----------------
            # Tap-granularity interleave: conv chunks of strip s+1 between
            # combine taps of strip s so no engine stream has long bursts
            # blocking another engine's PSUM drain.
            q_s = {}
            for fn in cf_chunks(0):
                fn()
            cf_math(0)
            idx_fetch(0)
            for s in range(NS):
                parts = cf_chunks(s + 1) if s + 1 < NS else [lambda: None] * 9
                gsched = {0: 0, 2: 1, 4: 2, 6: 3, 8: 4}
                for k in range(NT):
                    if k in gsched:
                        gather_taps(s, gsched[k])
                    parts[k]()
                    combine_tap(s, k)
                    if k >= 3:
                        tail(s, k - 3)
                if s + 1 < NS:
                    cf_math(s + 1)
                    idx_fetch(s + 1)
                for k in range(NT - 3, NT):
                    tail(s, k)
                strip_out(s)

    nc.compile()
    return nc


_NC_CACHE = {}


def _get_nc(dbg=0, reps=1):
    key = f"nc{dbg}_{reps}"
    if key not in _NC_CACHE:
        _NC_CACHE[key] = _build_nc(dbg, reps)
    return _NC_CACHE[key]


def _host_prep(lr_features, hr_features, est_w, est_b, off_w, off_b, dc_w, dc_b):
    """Build the 8 per-core input maps."""
    lr = np.asarray(lr_features, np.float32)
    hr = np.asarray(hr_features, np.float32)

    # weights as lhsT layouts
    w_est = np.transpose(np.asarray(est_w, np.float32), (1, 0, 2, 3)).reshape(
        2, C, 2 * C, 9)
    w_est = np.transpose(w_est, (0, 3, 1, 2)).astype(np.float16)       # [2,9,C,2C]
    w_off = np.transpose(np.asarray(off_w, np.float32), (1, 0, 2, 3)).reshape(
        2, C, 18, 9)
    w_off = np.transpose(w_off, (0, 3, 1, 2)).astype(np.float16)       # [2,9,C,18]
    w_dcT = np.transpose(np.asarray(dc_w, np.float32), (1, 0, 2, 3)).reshape(
        C, C, 9)
    w_dcT = np.transpose(w_dcT, (2, 0, 1)).astype(np.float16)          # [9,C,C]
    est_b2 = np.asarray(est_b, np.float32).reshape(2, C).T.copy()      # [C,2]
    off_b2 = np.asarray(off_b, np.float32).reshape(18, 1)
    dc_b2 = np.asarray(dc_b, np.float32).reshape(C, 1)

    in_maps = []
    for core in range(8):
        b, half = core // 2, core % 2
        h0 = 48 * half
        # conv input slices [C, 52, 98] fp16 (global rows h0-2 .. h0+49)
        def conv_slice(imgt):
            sl = np.zeros((C, HP, WP), np.float16)
            r0, r1 = h0 - 2, h0 + 50
            cr0, cr1 = max(r0, 0), min(r1, H)
            sl[:, cr0 - r0:cr1 - r0, 1:97] = imgt[:, cr0:cr1, :].astype(np.float16)
            return sl
        lr_sl = conv_slice(lr[b])
        hr_sl = conv_slice(hr[b])
        # dup2 pixel-major padded gather image
        pm = np.zeros((PW * PW + 212, C), np.float16)
        pm[:PW * PW] = np.pad(hr[b], ((0, 0), (2, 2), (2, 2))).reshape(
            C, PW * PW).T.astype(np.float16)
        dup = np.zeros((NDUP, 2, C), np.float16)
        dup[:, 0, :] = pm[:NDUP]
        dup[:, 1, :] = pm[100:NDUP + 100]
        # base coords, pixel-major: pixel i = j*128 + p
        i_idx = np.arange(NPIX)
        rows = (h0 + i_idx // 96).astype(np.float32)
        cols = (i_idx % 96).astype(np.float32)
        ky = (np.arange(NT) // 3 - 1).astype(np.float32)
        kx = (np.arange(NT) % 3 - 1).astype(np.float32)
        by = (rows[None, :] + ky[:, None])  # [9, NPIX]
        bx = (cols[None, :] + kx[:, None])
        # [C=128 partitions, 9, 36]: partition p, block j -> pixel j*128+p
        by_t = by.reshape(NT, NBLK, 128).transpose(2, 0, 1).copy()
        bx_t = bx.reshape(NT, NBLK, 128).transpose(2, 0, 1).copy()

        mask = np.broadcast_to(
            np.array([[0.0, 1.0]] if half == 0 else [[1.0, 0.0]], np.float32),
            (C, 2)).copy()
        in_maps.append({
            "lr_pad": lr_sl, "hr_cpad": hr_sl,
            "hr_dup": dup.reshape(-1),
            "w_est": w_est, "w_off": w_off, "w_dc": w_dcT,
            "est_b": est_b2, "off_b": off_b2, "dc_b": dc_b2,
            "base_y": by_t, "base_x": bx_t, "mask_e": mask,
        })
    return in_maps


def kernel(lr_features, hr_features, est_w, est_b, off_w, off_b, dc_w, dc_b):
    nc = _get_nc()
    in_maps = _host_prep(lr_features, hr_features, est_w, est_b,
                         off_w, off_b, dc_w, dc_b)
    res = run_bass_kernel_spmd(nc, in_maps, core_ids=list(range(8))).results
    out = np.empty((B, C, H, W), np.float32)
    for core in range(8):
        b, half = core // 2, core % 2
        h0 = 48 * half
        o = res[core]["out"]  # [C, NPIX]
        out[b, :, h0:h0 + 48, :] = o.reshape(C, 48, 96)
    return out
